# revision 21
# baseline (speedup 1.0000x reference)
"""Trainium2 Bass kernel for nn_AttentionBlock (GroupNorm + MHA + proj + residual).

Sharding: data-parallel over batch; 8 batches -> 8 NeuronCores, one batch each.

Per-core layout (c=512 channels, t=1024 spatial, H=8 heads, ch=64):
  - x, h kept as [c-on-partitions, t] (4 tiles of [128, 1024])
  - GroupNorm group-reduction done with two tiny mask matmuls on the PE
    (no cross-partition DMA shuffles)
  - q,k per head PAIR: qk[pi] = [p, {q,k}, t], partitions 0-63 = head 2pi,
    64-127 = head 2pi+1 (w_qkv rows permuted host-side; q pre-scaled by
    1/sqrt(ch) = 0.125, exact).
  - v computed directly transposed: vT[t, o_v] via matmul(lhsT=h, rhs=WvT), with
    a ones column per head -> the AV matmul also emits the softmax denominator Z
  - scores computed transposed: S^T[s, t] = k^T q, so exp(S^T) (ACT, psum->sbuf)
    feeds the AV matmul with s as the contraction dim; softmax skips the
    max-subtraction (scores are ~N(0,1), exp is safe in fp32)
  - Z normalization: reciprocal of the Z row ([1, 1024], DVE), broadcast across
    64 partitions with a stride-0 SBUF->SBUF DMA, fused into the normalization
    multiply.  No DRAM round trips.
  - proj: pairs 0,1 accumulate via DVE adds spread through the attention
    pipeline; pairs 2,3 share one PSUM accumulation group in the tail.
  - input DMAs: x tiles split across 4 queues (sync/scalar/gpsimd/tensor) ahead
    of all weights; the vector queue stays DMA-free so GroupNorm starts ASAP.
All matmuls run in fp32r (same PE rate as bf16 here; ~1.5e-4 max rel err).
"""

import numpy as np

B, C, HW, T = 8, 512, 32, 1024
H, CH = 8, 64
G, GS = 32, 16
EPS = 1e-5
NCORES = 8

_CACHE = {}
TRACE = False  # test harness can set kernel.TRACE = True to get a profile


def _install_ntff_hook():
    import sys, types
    if 'antenv.axon_hooks' in sys.modules:
        return
    mod = types.ModuleType('antenv.axon_hooks')
    state = {'hook': None}
    mod.set_axon_ntff_profile_hook = lambda h: state.__setitem__('hook', h)
    mod.get_axon_ntff_profile_hook = lambda: state['hook']
    sys.modules['antenv.axon_hooks'] = mod
    import antenv
    antenv.axon_hooks = mod
    try:
        from trn_agent_boot.trn_boot import _ntff_profile_via_ctypes
        mod.set_axon_ntff_profile_hook(_ntff_profile_via_ctypes('/opt/axon/libaxon_pjrt.so'))
    except Exception:
        pass


def _split_multi_waits(nc, max_waits=1):
    """This container's walrus supports only one sync wait per instruction; move
    extra waits onto same-engine no-ops inserted just before the instruction."""
    import concourse.mybir as mybir
    for f in nc.m.functions:
        for bb in f.blocks:
            insts = bb.instructions
            out = []
            changed = False
            for inst in insts:
                si = inst.sync_info
                waits = list(si.on_wait) if si is not None and si.on_wait else []
                if len(waits) > max_waits:
                    changed = True
                    for j, w in enumerate(waits[:-max_waits]):
                        out.append(mybir.InstNoOp(
                            name=f"{inst.name}-ws{j}",
                            sync_info=mybir.SyncInfo(on_wait=[w], on_update=[]),
                            bass_nofuse=True,
                            engine=inst.engine,
                        ))
                    inst.sync_info = mybir.SyncInfo(
                        on_wait=waits[-max_waits:],
                        on_update=list(si.on_update) if si.on_update else [],
                    )
                out.append(inst)
            if changed:
                bb.instructions = out


def _build_nc():
    import concourse.bass as bass
    import concourse.tile as tile
    import concourse.mybir as mybir

    f32 = mybir.dt.float32
    f32r = mybir.dt.float32r
    Alu = mybir.AluOpType
    Act = mybir.ActivationFunctionType

    nc = bass.Bass()

    xin = nc.dram_tensor("xin", [C, T], f32, kind="ExternalInput")
    wqkT = nc.dram_tensor("wqkT", [C, 1024], f32r, kind="ExternalInput")
    wvT = nc.dram_tensor("wvT", [C, C], f32r, kind="ExternalInput")
    wpT = nc.dram_tensor("wpT", [C, C], f32r, kind="ExternalInput")
    bqk = nc.dram_tensor("bqk", [1024], f32, kind="ExternalInput")
    bv = nc.dram_tensor("bv", [C], f32, kind="ExternalInput")
    bp = nc.dram_tensor("bp", [C], f32, kind="ExternalInput")
    gam = nc.dram_tensor("gam", [C], f32, kind="ExternalInput")
    bet = nc.dram_tensor("bet", [C], f32, kind="ExternalInput")
    onesc = nc.dram_tensor("onesc", [128, 8], f32r, kind="ExternalInput")
    maskA = nc.dram_tensor("maskA", [128, 8], f32r, kind="ExternalInput")
    maskB = nc.dram_tensor("maskB", [8, 128], f32r, kind="ExternalInput")
    outd = nc.dram_tensor("outd", [C, T], f32, kind="ExternalOutput")

    with tile.TileContext(nc) as tc:
        with tc.tile_pool(name="const", bufs=1) as const, \
             tc.tile_pool(name="big", bufs=1) as big, \
             tc.tile_pool(name="qpp", bufs=2) as qpp, \
             tc.tile_pool(name="kpp", bufs=4) as kpp, \
             tc.tile_pool(name="esp", bufs=4) as esp, \
             tc.tile_pool(name="accp", bufs=1) as accp, \
             tc.tile_pool(name="zp", bufs=2) as zp, \
             tc.tile_pool(name="gn", bufs=2) as gn, \
             tc.tile_pool(name="ps", bufs=2, space="PSUM") as ps, \
             tc.tile_pool(name="dram", bufs=2, space="DRAM") as dram:

            # ---- input DMAs.  x tiles first, one per queue, so GroupNorm can
            # start ~5us in; weights follow on the same queues.  The vector
            # queue issues no DMAs (it runs GroupNorm + evacuations). ----
            xt = [big.tile([128, 1024], f32, tag=f"x{ci}", name=f"x{ci}") for ci in range(4)]
            xr = xin.rearrange("(ci p) t -> p ci t", p=128)
            nc.sync.dma_start(out=xt[0], in_=xr[:, 0, :])
            nc.scalar.dma_start(out=xt[1], in_=xr[:, 1, :])
            nc.gpsimd.dma_start(out=xt[2], in_=xr[:, 2, :])
            nc.sync.dma_start(out=xt[3][:, 0:512], in_=xr[:, 3, 0:512])
            nc.scalar.dma_start(out=xt[3][:, 512:1024], in_=xr[:, 3, 512:1024])

            # tiny GN constants right behind x on sync/scalar (land ~x-time)
            gam_t = const.tile([128, 4], f32)
            nc.sync.dma_start(out=gam_t, in_=gam.rearrange("(ci p) -> p ci", p=128))
            bet_t = const.tile([128, 4], f32)
            nc.sync.dma_start(out=bet_t, in_=bet.rearrange("(ci p) -> p ci", p=128))
            mA = const.tile([128, 8], f32r)
            nc.scalar.dma_start(out=mA, in_=maskA[:, :])
            mB = const.tile([8, 128], f32r)
            nc.scalar.dma_start(out=mB, in_=maskB[:, :])
            eps_t = const.tile([128, 1], f32)
            nc.vector.memset(eps_t, EPS)

            # v weights next (v runs first after GN)
            wvT_t = const.tile([128, 4, 512], f32r)
            wvr = wvT.rearrange("(ci p) o -> p ci o", p=128)
            nc.sync.dma_start(out=wvT_t[:, 0:2, :], in_=wvr[:, 0:2, :])
            nc.scalar.dma_start(out=wvT_t[:, 2:4, :], in_=wvr[:, 2:4, :])
            # qk weights
            wqkT_t = const.tile([128, 4, 1024], f32r)
            wqr = wqkT.rearrange("(ci p) o -> p ci o", p=128)
            weng = [nc.sync, nc.scalar, nc.gpsimd, nc.gpsimd]
            for ci in range(4):
                weng[ci].dma_start(out=wqkT_t[:, ci, :], in_=wqr[:, ci, :])
            # small biases + ones
            bqk_t = const.tile([128, 8], f32)
            nc.gpsimd.dma_start(out=bqk_t, in_=bqk.rearrange("(oi p) -> p oi", p=128))
            bv_b = const.tile([128, 512], f32)
            nc.gpsimd.dma_start(out=bv_b, in_=bass.AP(
                tensor=bv, offset=0, ap=[[0, 128], [1, 512]]))
            bp_t = const.tile([128, 4], f32)
            nc.sync.dma_start(out=bp_t, in_=bp.rearrange("(ci p) -> p ci", p=128))
            # proj weights last (needed latest)
            wpT_t = const.tile([128, 4, 512], f32r)
            wpr = wpT.rearrange("(ci p) o -> p ci o", p=128)
            nc.sync.dma_start(out=wpT_t[:, 0:2, :], in_=wpr[:, 0:2, :])
            nc.scalar.dma_start(out=wpT_t[:, 2:4, :], in_=wpr[:, 2:4, :])

            # ---- GroupNorm, pipelined per 128-channel tile ----
            # each ci tile holds 8 complete groups (16 channels each), so the
            # whole stats -> group-reduce -> normalize chain runs per ci as
            # its x DMA lands, overlapping the remaining x/weight DMAs.
            # The tiny mask matmuls use PSUM tag "a" (idle until attention).
            ht = [big.tile([128, 1024], f32r, tag=f"h{ci}", name=f"h{ci}") for ci in range(4)]
            chmv = gn.tile([128, 4, 2], f32)
            s2ch = gn.tile([128, 4, 2], f32r)
            gf = gn.tile([8, 4, 2], f32r)
            mg = gn.tile([8, 4], f32)
            vg = gn.tile([8, 4], f32)
            m2 = gn.tile([8, 4], f32)
            scl = gn.tile([128, 4], f32)
            sht = gn.tile([128, 4], f32)
            pg = ps.tile([128, 1024], f32, tag="a", name="pgn")
            pc = ps.tile([128, 1024], f32, tag="a", name="pgc")
            for ci in range(4):
                st = gn.tile([128, 2, 6], f32, tag="st")
                xv = xt[ci].rearrange("p (n f) -> p n f", f=512)
                for sub in range(2):
                    nc.vector.bn_stats(out=st[:, sub, :], in_=xv[:, sub, :])
                nc.vector.bn_aggr(out=chmv[:, ci, :], in_=st)
                # per-channel [mean, var+mean^2] as f32r for the mask matmul
                nc.vector.tensor_copy(out=s2ch[:, ci, 0:1], in_=chmv[:, ci, 0:1])
                t1 = gn.tile([128, 4], f32, tag="t1")
                nc.vector.tensor_mul(out=t1[:, ci:ci + 1], in0=chmv[:, ci, 0:1],
                                     in1=chmv[:, ci, 0:1])
                nc.vector.tensor_add(out=s2ch[:, ci, 1:2], in0=t1[:, ci:ci + 1],
                                     in1=chmv[:, ci, 1:2])
                # group sums for this tile's 8 groups
                nc.tensor.matmul(pg[0:8, 2 * ci:2 * ci + 2], lhsT=mA,
                                 rhs=s2ch[:, ci, :], start=True, stop=True)
                nc.vector.tensor_scalar_mul(out=mg[:, ci:ci + 1],
                                            in0=pg[0:8, 2 * ci:2 * ci + 1],
                                            scalar1=1.0 / GS)
                nc.vector.tensor_scalar_mul(out=vg[:, ci:ci + 1],
                                            in0=pg[0:8, 2 * ci + 1:2 * ci + 2],
                                            scalar1=1.0 / GS)
                nc.vector.tensor_mul(out=m2[:, ci:ci + 1], in0=mg[:, ci:ci + 1],
                                     in1=mg[:, ci:ci + 1])
                nc.vector.tensor_sub(out=vg[:, ci:ci + 1], in0=vg[:, ci:ci + 1],
                                     in1=m2[:, ci:ci + 1])
                # rstd = 1/sqrt(vg + eps)
                nc.scalar.activation(out=vg[:, ci:ci + 1], in_=vg[:, ci:ci + 1],
                                     func=Act.Sqrt, bias=eps_t[:8], scale=1.0)
                nc.vector.reciprocal(out=vg[:, ci:ci + 1], in_=vg[:, ci:ci + 1])
                nc.vector.tensor_copy(out=gf[:, ci, 0:1], in_=mg[:, ci:ci + 1])
                nc.vector.tensor_copy(out=gf[:, ci, 1:2], in_=vg[:, ci:ci + 1])
                # broadcast group stats back to the tile's 128 channels
                nc.tensor.matmul(pc[:, 2 * ci:2 * ci + 2], lhsT=mB,
                                 rhs=gf[:, ci, :], start=True, stop=True)
                nc.vector.tensor_mul(out=scl[:, ci:ci + 1], in0=gam_t[:, ci:ci + 1],
                                     in1=pc[:, 2 * ci + 1:2 * ci + 2])
                nc.vector.tensor_mul(out=sht[:, ci:ci + 1], in0=scl[:, ci:ci + 1],
                                     in1=pc[:, 2 * ci:2 * ci + 1])
                nc.vector.tensor_sub(out=sht[:, ci:ci + 1], in0=bet_t[:, ci:ci + 1],
                                     in1=sht[:, ci:ci + 1])
                # h = x * scl + sht   (f32r, feeds matmuls)
                nc.vector.tensor_scalar(out=ht[ci], in0=xt[ci],
                                        scalar1=scl[:, ci:ci + 1], scalar2=sht[:, ci:ci + 1],
                                        op0=Alu.mult, op1=Alu.add)

            # ---- vT[t, o_v] first (needs only h), so attention can start as
            # soon as the first qkv pair lands ----
            # cols 64:128 of each [t, head] block are all-ones: the AV matmul
            # then emits the softmax denominator Z broadcast across partitions
            # 64:128 of its PSUM tile (matmul cost only depends on N).
            vTa = big.tile([128, 8, 8, 128], f32r)  # [t_part, ti, hd, ch+ones]
            nc.gpsimd.memset(vTa[:, :, :, 64:128].bitcast(f32), 1.0)
            for tp in range(4):
                pv = ps.tile([128, 1024], f32, tag="s", name=f"pv{tp}")
                for ci in range(4):
                    for half in range(2):
                        ti = 2 * tp + half
                        nc.tensor.matmul(pv[:, half * 512:(half + 1) * 512],
                                         lhsT=ht[ci][:, ti * 128:(ti + 1) * 128],
                                         rhs=wvT_t[:, ci, :], start=(ci == 0), stop=(ci == 3))
                nc.vector.tensor_add(
                    out=vTa[:, 2 * tp:2 * tp + 2, :, 0:64],
                    in0=pv.rearrange("p (t h c) -> p t h c", t=2, h=8),
                    in1=bass.AP(tensor=bv_b.tensor, offset=bv_b.offset,
                                ap=[list(bv_b.ap[0])] + [[0, 2]] + [[64, 8], [1, 64]]))

            # ---- q,k for one head pair.  q stays paired [q_A; q_B] on 128
            # partitions; each head's k is zero-padded to the full 128
            # partitions so the score matmul runs with K=128 at the fast PE
            # rate (the zero rows multiply the other head's q to nothing;
            # matmul cost only depends on N). ----
            qp = [None] * 4
            kpad = [None] * 8

            def qkv_pair(pi):
                qp[pi] = qpp.tile([128, 1024], f32r, tag="qp", name=f"qp{pi}")
                kpad[2 * pi] = kpp.tile([128, 1024], f32r, tag="kp", name=f"kp{2*pi}")
                kpad[2 * pi + 1] = kpp.tile([128, 1024], f32r, tag="kp", name=f"kp{2*pi+1}")
                nc.gpsimd.memset(kpad[2 * pi][64:128, :].bitcast(f32), 0.0)
                nc.gpsimd.memset(kpad[2 * pi + 1][0:64, :].bitcast(f32), 0.0)
                for side in range(2):
                    oi = side * 4 + pi
                    pqk = ps.tile([128, 1024], f32, tag="s", name=f"pqk{oi}")
                    for ci in range(4):
                        for ni in range(2):
                            nc.tensor.matmul(pqk[:, ni * 512:(ni + 1) * 512],
                                             lhsT=wqkT_t[:, ci, oi * 128:(oi + 1) * 128],
                                             rhs=ht[ci][:, ni * 512:(ni + 1) * 512],
                                             start=(ci == 0), stop=(ci == 3))
                    if side == 0:
                        nc.vector.tensor_scalar_add(out=qp[pi], in0=pqk,
                                                    scalar1=bqk_t[:, oi:oi + 1])
                    else:
                        nc.vector.tensor_scalar_add(
                            out=kpad[2 * pi][0:64, :], in0=pqk[0:64, :],
                            scalar1=bqk_t[0:64, oi:oi + 1])
                        nc.vector.tensor_scalar_add(
                            out=kpad[2 * pi + 1][64:128, :], in0=pqk[64:128, :],
                            scalar1=bqk_t[64:128, oi:oi + 1])

            # ---- attention (one head pair at a time) ----
            at_ = [None] * 4   # a[c(hd-major), t] per pair
            acc = [accp.tile([128, 1024], f32, tag=f"acc{oi}", name=f"acc{oi}")
                   for oi in range(4)]
            pa_all = [None] * 4

            def attn_core(pi):
                pa = [ps.tile([128, 1024], f32, tag="a", name=f"pa{pi}_{i}") for i in range(2)]
                pa_all[pi] = pa
                for si in range(8):
                    pss = [ps.tile([128, 1024], f32, tag="s", name=f"pss{pi}_{si}_{i}")
                           for i in range(2)]
                    for half in range(2):
                        for ni in range(2):
                            nc.tensor.matmul(
                                pss[half][:, ni * 512:(ni + 1) * 512],
                                lhsT=kpad[2 * pi + half][:, si * 128:(si + 1) * 128],
                                rhs=qp[pi][:, ni * 512:(ni + 1) * 512],
                                start=True, stop=True)
                    ess = []
                    for half in range(2):
                        es = esp.tile([128, 1024], f32r, tag="es")
                        nc.scalar.activation(out=es, in_=pss[half], func=Act.Exp)
                        ess.append(es)
                    for half in range(2):
                        hd = 2 * pi + half
                        for ni in range(2):
                            nc.tensor.matmul(pa[half][:, ni * 512:(ni + 1) * 512],
                                             lhsT=vTa[:, si, hd, :],
                                             rhs=ess[half][:, ni * 512:(ni + 1) * 512],
                                             start=(si == 0), stop=(si == 7))

            def divide(pi, tail=False):
                """at_[pi] = AV / Z.  Z arrives pre-broadcast on partitions
                64:128 of the AV PSUM tile (ones-columns in vTa).
                Exact DVE reciprocal costs ~4.7ns/free-elem, so the fast path
                reshapes Z to [128, 8] via a DRAM round trip (off the critical
                path; an early copy releases the PSUM banks).  The tail pair
                instead computes 1/Z = Exp(-Ln(Z)) on the then-idle ACT engine
                (ln+exp share an activation table with the attention exps),
                split by t-halves so proj can chase it."""
                at_[pi] = big.tile([128, 1024], f32r, tag=f"at{pi}", name=f"at{pi}")
                pa = pa_all[pi]
                if tail:
                    for ni in range(2):
                        sl = slice(ni * 512, (ni + 1) * 512)
                        for half in range(2):
                            lz = zp.tile([64, 1024], f32, tag="lz")
                            nc.scalar.activation(out=lz[:, sl], in_=pa[half][64:128, sl],
                                                 func=Act.Ln)
                            rzb = zp.tile([64, 1024], f32, tag="rz")
                            nc.scalar.activation(out=rzb[:, sl], in_=lz[:, sl],
                                                 func=Act.Exp, scale=-1.0)
                            nc.vector.tensor_tensor(
                                out=at_[pi][half * 64:half * 64 + 64, sl],
                                in0=pa[half][0:64, sl], in1=rzb[:, sl], op=Alu.mult)
                    return
                aus = []
                for half in range(2):
                    au = zp.tile([65, 1024], f32, tag="au")
                    nc.vector.tensor_copy(out=au, in_=pa[half][0:65, :])
                    aus.append(au)
                for half in range(2):
                    eng = nc.sync if half == 0 else nc.gpsimd
                    zd = dram.tile([1, 1024], f32, tag="zd")
                    eng.dma_start(out=zd, in_=aus[half][64:65, :])
                    zq = zp.tile([128, 8], f32, tag="zq")
                    eng.dma_start(out=zq, in_=bass.AP(
                        tensor=zd.tensor, offset=zd.offset, ap=[[8, 128], [1, 8]]))
                    nc.vector.reciprocal(out=zq, in_=zq)
                    zd2 = dram.tile([1, 1024], f32, tag="zd2")
                    eng.dma_start(
                        out=bass.AP(tensor=zd2.tensor, offset=zd2.offset,
                                    ap=[[8, 128], [1, 8]]), in_=zq)
                    zb = zp.tile([64, 1024], f32, tag="zb")
                    eng.dma_start(out=zb, in_=bass.AP(
                        tensor=zd2.tensor, offset=zd2.offset, ap=[[0, 64], [1, 1024]]))
                    nc.vector.tensor_tensor(
                        out=at_[pi][half * 64:half * 64 + 64, :],
                        in0=aus[half][0:64, :], in1=zb, op=Alu.mult)

            def proj_part(pi, tag="a"):
                # contribution of channel block pi to every output block
                for oi in range(4):
                    pp = ps.tile([128, 1024], f32, tag=tag, name=f"pp{pi}_{oi}")
                    for ni in range(2):
                        nc.tensor.matmul(pp[:, ni * 512:(ni + 1) * 512],
                                         lhsT=wpT_t[:, pi, oi * 128:(oi + 1) * 128],
                                         rhs=at_[pi][:, ni * 512:(ni + 1) * 512],
                                         start=True, stop=True)
                    if pi == 0:
                        nc.vector.scalar_tensor_tensor(
                            out=acc[oi], in0=pp, scalar=bp_t[:, oi:oi + 1],
                            in1=xt[oi], op0=Alu.add, op1=Alu.add)
                    else:
                        nc.vector.tensor_add(out=acc[oi], in0=acc[oi], in1=pp)

            def proj_tail():
                """pair 3 only: per output block 2 matmuls -> add -> DMA out,
                pipelined behind the t-half-split divide(3)."""
                oeng = [nc.sync, nc.scalar, nc.gpsimd, nc.sync]
                outr = outd.rearrange("(ci p) t -> p ci t", p=128)
                for oi in range(4):
                    pp = ps.tile([128, 1024], f32, tag=("a" if oi >= 2 else "s"),
                                 name=f"ppt_{oi}")
                    for ni in range(2):
                        nc.tensor.matmul(pp[:, ni * 512:(ni + 1) * 512],
                                         lhsT=wpT_t[:, 3, oi * 128:(oi + 1) * 128],
                                         rhs=at_[3][:, ni * 512:(ni + 1) * 512],
                                         start=True, stop=True)
                    nc.vector.tensor_add(out=acc[oi], in0=acc[oi], in1=pp)
                    oeng[oi].dma_start(out=outr[:, oi, :], in_=acc[oi])

            qkv_pair(0)
            attn_core(0)
            divide(0)
            qkv_pair(1)
            attn_core(1)
            divide(1)
            qkv_pair(2)
            proj_part(0)
            attn_core(2)
            divide(2)
            qkv_pair(3)
            proj_part(1)
            proj_part(2, tag="s")
            attn_core(3)
            divide(3, tail=True)
            proj_tail()

    _split_multi_waits(nc)
    return nc


def _prep_host(x, gamma, beta, w_qkv, b_qkv, w_proj, b_proj):
    """Host-side weight permutation/pre-scaling + per-core input maps."""
    x = np.ascontiguousarray(x, dtype=np.float32).reshape(B, C, T)
    scale2 = 1.0 / np.sqrt(CH)  # folded into q (exact: 0.125 is a power of two)

    w = np.asarray(w_qkv, dtype=np.float32).reshape(H, 3, CH, C)
    bq = np.asarray(b_qkv, dtype=np.float32).reshape(H, 3, CH)
    wq = w[:, 0] * scale2          # [hd, 64, c]
    wk = w[:, 1]
    wv = w[:, 2]
    # o-block order: 4 q-blocks (one per head pair: [q_{2i}; q_{2i+1}]), 4 k-blocks
    qcols = wq.reshape(4, 128, C).transpose(2, 0, 1)            # [c, pi, 128]
    kcols = wk.reshape(4, 128, C).transpose(2, 0, 1)
    wqkT_host = np.ascontiguousarray(
        np.concatenate([qcols, kcols], axis=1).reshape(C, 1024))
    bqk_host = np.ascontiguousarray(np.concatenate(
        [(bq[:, 0] * scale2).reshape(4, 128), bq[:, 1].reshape(4, 128)], axis=0
    ).reshape(1024))
    wvT_host = np.ascontiguousarray(wv.transpose(2, 0, 1).reshape(C, C))
    bv_host = np.ascontiguousarray(bq[:, 2].reshape(C))
    wpT_host = np.ascontiguousarray(np.asarray(w_proj, dtype=np.float32).T)
    ones_host = np.ones((128, 8), np.float32)
    maskA_host = np.zeros((128, 8), np.float32)
    for p in range(128):
        maskA_host[p, p // 16] = 1.0
    maskB_host = np.ascontiguousarray(maskA_host.T)

    common = {
        "wqkT": wqkT_host, "wvT": wvT_host, "wpT": wpT_host,
        "bqk": bqk_host, "bv": bv_host,
        "bp": np.ascontiguousarray(np.asarray(b_proj, np.float32)),
        "gam": np.ascontiguousarray(np.asarray(gamma, np.float32)),
        "bet": np.ascontiguousarray(np.asarray(beta, np.float32)),
        "onesc": ones_host, "maskA": maskA_host, "maskB": maskB_host,
    }
    return [dict(common, xin=np.ascontiguousarray(x[b])) for b in range(B)]


def kernel(x, gamma, beta, w_qkv, b_qkv, w_proj, b_proj):
    from concourse.bass_utils import run_bass_kernel_spmd

    if "nc" not in _CACHE:
        _CACHE["nc"] = _build_nc()
    nc = _CACHE["nc"]

    in_maps = _prep_host(x, gamma, beta, w_qkv, b_qkv, w_proj, b_proj)
    kwargs = {}
    if TRACE:
        _install_ntff_hook()
        kwargs["trace"] = True
    res = run_bass_kernel_spmd(nc, in_maps, core_ids=list(range(NCORES)), **kwargs)
    if TRACE:
        _CACHE["last_result"] = res
    out = np.stack([r["outd"] for r in res.results], axis=0)
    return out.reshape(B, C, HW, HW)


# revision 24
# speedup vs baseline: 1.1734x; 1.1734x over previous
"""Trainium2 Bass kernel for nn_AttentionBlock (GroupNorm + MHA + proj + residual).

Sharding: data-parallel over batch; 8 batches -> 8 NeuronCores, one batch each.

Per-core layout (c=512 channels, t=1024 spatial, H=8 heads, ch=64):
  - x kept as [c-on-partitions, t] (4 tiles of [128, 1024], fp32)
  - GroupNorm pipelined per 128-channel tile (each tile holds 8 complete
    groups), group reduction via two tiny mask matmuls on the PE
  - h, v/vT, softmax weights (es), attention output (at) and the qkv/proj
    weights are fp8 e4m3; qkv/v/AV/proj matmuls run in DoubleRow perf mode
    (2 stacked K-tiles per pass -> half the matmul count).  Scores stay f32r
    (contraction is only 64 deep -> DoubleRow gains nothing, keeps precision).
  - q,k per head PAIR: q paired [q_A; q_B] on 128 partitions; per-head k
    zero-padded to 128 partitions (f32r, cost only depends on N)
  - scores computed transposed: S^T[s, t] = k^T q; exp(S^T - ln16) on ACT
    writes fp8 es directly (the 1/16 keeps es in e4m3 range and cancels in the
    softmax normalization); si-pairs share one es tile = the two DoubleRow
    K-tiles of the AV matmul
  - vT carries 64 all-ones columns -> the AV matmul emits the softmax
    denominator Z broadcast on partitions 64:128 of its PSUM tile
  - Z normalization: pairs 0-2 reshape Z to [128, 8] via a DRAM round trip for
    a cheap exact DVE reciprocal (off the critical path; an early copy releases
    the PSUM banks).  Pair 3 computes 1/Z = Exp(-Ln(Z)) on the then-idle ACT
    engine (ln+exp share an activation table), split by t-halves so the tail
    proj can chase it.
  - proj: pairs {0,1} and {2,3} are DoubleRow K-tile pairs; {0,1} runs between
    attn cores 2 and 3, {2,3} in the tail, each + bias/residual on DVE.
  - input DMAs: x tiles split across the 3 DMA-capable queues ahead of all
    weights; the vector queue stays DMA-free so GroupNorm starts ASAP.
"""

import numpy as np

B, C, HW, T = 8, 512, 32, 1024
H, CH = 8, 64
G, GS = 32, 16
EPS = 1e-5
NCORES = 8
LOG16 = float(np.log(16.0))

_CACHE = {}
TRACE = False  # test harness can set kernel.TRACE = True to get a profile


def _install_ntff_hook():
    import sys, types
    if 'antenv.axon_hooks' in sys.modules:
        return
    mod = types.ModuleType('antenv.axon_hooks')
    state = {'hook': None}
    mod.set_axon_ntff_profile_hook = lambda h: state.__setitem__('hook', h)
    mod.get_axon_ntff_profile_hook = lambda: state['hook']
    sys.modules['antenv.axon_hooks'] = mod
    import antenv
    antenv.axon_hooks = mod
    try:
        from trn_agent_boot.trn_boot import _ntff_profile_via_ctypes
        mod.set_axon_ntff_profile_hook(_ntff_profile_via_ctypes('/opt/axon/libaxon_pjrt.so'))
    except Exception:
        pass


def _split_multi_waits(nc, max_waits=1):
    """This container's walrus supports only one sync wait per instruction; move
    extra waits onto same-engine no-ops inserted just before the instruction."""
    import concourse.mybir as mybir
    for f in nc.m.functions:
        for bb in f.blocks:
            insts = bb.instructions
            out = []
            changed = False
            for inst in insts:
                si = inst.sync_info
                waits = list(si.on_wait) if si is not None and si.on_wait else []
                if len(waits) > max_waits:
                    changed = True
                    for j, w in enumerate(waits[:-max_waits]):
                        out.append(mybir.InstNoOp(
                            name=f"{inst.name}-ws{j}",
                            sync_info=mybir.SyncInfo(on_wait=[w], on_update=[]),
                            bass_nofuse=True,
                            engine=inst.engine,
                        ))
                    inst.sync_info = mybir.SyncInfo(
                        on_wait=waits[-max_waits:],
                        on_update=list(si.on_update) if si.on_update else [],
                    )
                out.append(inst)
            if changed:
                bb.instructions = out


def _build_nc():
    import concourse.bass as bass
    import concourse.tile as tile
    import concourse.mybir as mybir

    f32 = mybir.dt.float32
    f32r = mybir.dt.float32r
    f8 = mybir.dt.float8e4
    Alu = mybir.AluOpType
    Act = mybir.ActivationFunctionType
    DR = mybir.MatmulPerfMode.DoubleRow

    nc = bass.Bass()

    xin = nc.dram_tensor("xin", [C, T], f32, kind="ExternalInput")
    wqkT = nc.dram_tensor("wqkT", [C, 1024], f8, kind="ExternalInput")
    wvT8 = nc.dram_tensor("wvT8", [C, C], f8, kind="ExternalInput")
    wpT8 = nc.dram_tensor("wpT8", [C, C], f8, kind="ExternalInput")
    bqk = nc.dram_tensor("bqk", [1024], f32, kind="ExternalInput")
    bv = nc.dram_tensor("bv", [C], f32, kind="ExternalInput")
    bp = nc.dram_tensor("bp", [C], f32, kind="ExternalInput")
    gam = nc.dram_tensor("gam", [C], f32, kind="ExternalInput")
    bet = nc.dram_tensor("bet", [C], f32, kind="ExternalInput")
    maskA = nc.dram_tensor("maskA", [128, 8], f32r, kind="ExternalInput")
    maskB = nc.dram_tensor("maskB", [8, 128], f32r, kind="ExternalInput")
    outd = nc.dram_tensor("outd", [C, T], f32, kind="ExternalOutput")

    with tile.TileContext(nc) as tc:
        with tc.tile_pool(name="const", bufs=1) as const, \
             tc.tile_pool(name="big", bufs=1) as big, \
             tc.tile_pool(name="qpp", bufs=2) as qpp, \
             tc.tile_pool(name="kpp", bufs=4) as kpp, \
             tc.tile_pool(name="esp", bufs=6) as esp, \
             tc.tile_pool(name="accp", bufs=1) as accp, \
             tc.tile_pool(name="zp", bufs=2) as zp, \
             tc.tile_pool(name="gn", bufs=2) as gn, \
             tc.tile_pool(name="ps", bufs=2, space="PSUM") as ps, \
             tc.tile_pool(name="dram", bufs=2, space="DRAM") as dram:

            # ---- input DMAs.  x tiles first so GroupNorm can start ~5us in;
            # weights follow on the same queues.  The vector queue issues no
            # DMAs (it runs GroupNorm + evacuations). ----
            xt = [big.tile([128, 1024], f32, tag=f"x{ci}", name=f"x{ci}") for ci in range(4)]
            xr = xin.rearrange("(ci p) t -> p ci t", p=128)
            nc.sync.dma_start(out=xt[0], in_=xr[:, 0, :])
            nc.scalar.dma_start(out=xt[1], in_=xr[:, 1, :])
            nc.gpsimd.dma_start(out=xt[2], in_=xr[:, 2, :])
            nc.sync.dma_start(out=xt[3][:, 0:512], in_=xr[:, 3, 0:512])
            nc.scalar.dma_start(out=xt[3][:, 512:1024], in_=xr[:, 3, 512:1024])

            # tiny GN constants right behind x on sync/scalar (land ~x-time)
            gam_t = const.tile([128, 4], f32)
            nc.sync.dma_start(out=gam_t, in_=gam.rearrange("(ci p) -> p ci", p=128))
            bet_t = const.tile([128, 4], f32)
            nc.sync.dma_start(out=bet_t, in_=bet.rearrange("(ci p) -> p ci", p=128))
            mA = const.tile([128, 8], f32r)
            nc.scalar.dma_start(out=mA, in_=maskA[:, :])
            mB = const.tile([8, 128], f32r)
            nc.scalar.dma_start(out=mB, in_=maskB[:, :])
            eps_t = const.tile([128, 1], f32)
            nc.vector.memset(eps_t, EPS)
            nl16_t = const.tile([128, 1], f32)
            nc.vector.memset(nl16_t, -LOG16)

            # v weights next (v runs first after GN)
            wvT_t = const.tile([128, 4, 512], f8)
            wvr = wvT8.rearrange("(ci p) o -> p ci o", p=128)
            nc.sync.dma_start(out=wvT_t[:, 0:2, :], in_=wvr[:, 0:2, :])
            nc.scalar.dma_start(out=wvT_t[:, 2:4, :], in_=wvr[:, 2:4, :])
            # qk weights
            wqkT_t = const.tile([128, 4, 1024], f8)
            wqr = wqkT.rearrange("(ci p) o -> p ci o", p=128)
            weng = [nc.sync, nc.scalar, nc.gpsimd, nc.gpsimd]
            for ci in range(4):
                weng[ci].dma_start(out=wqkT_t[:, ci, :], in_=wqr[:, ci, :])
            # small biases
            bqk_t = const.tile([128, 8], f32)
            nc.gpsimd.dma_start(out=bqk_t, in_=bqk.rearrange("(oi p) -> p oi", p=128))
            bv_b = const.tile([128, 512], f32)
            nc.gpsimd.dma_start(out=bv_b, in_=bass.AP(
                tensor=bv, offset=0, ap=[[0, 128], [1, 512]]))
            bp_t = const.tile([128, 4], f32)
            nc.sync.dma_start(out=bp_t, in_=bp.rearrange("(ci p) -> p ci", p=128))
            # proj weights last (needed latest)
            wpT_t = const.tile([128, 4, 512], f8)
            wpr = wpT8.rearrange("(ci p) o -> p ci o", p=128)
            nc.sync.dma_start(out=wpT_t[:, 0:2, :], in_=wpr[:, 0:2, :])
            nc.scalar.dma_start(out=wpT_t[:, 2:4, :], in_=wpr[:, 2:4, :])

            # ---- GroupNorm, pipelined per 128-channel tile ----
            # each ci tile holds 8 complete groups (16 channels each), so the
            # whole stats -> group-reduce -> normalize chain runs per ci as
            # its x DMA lands, overlapping the remaining x/weight DMAs.
            # The tiny mask matmuls use PSUM tag "a" (idle until attention).
            ht = big.tile([128, 4, 1024], f8, tag="h", name="h")
            chmv = gn.tile([128, 4, 2], f32)
            s2ch = gn.tile([128, 4, 2], f32r)
            gf = gn.tile([8, 4, 2], f32r)
            mg = gn.tile([8, 4], f32)
            vg = gn.tile([8, 4], f32)
            m2 = gn.tile([8, 4], f32)
            scl = gn.tile([128, 4], f32)
            sht = gn.tile([128, 4], f32)
            pg = ps.tile([128, 1024], f32, tag="a", name="pgn")
            pc = ps.tile([128, 1024], f32, tag="a", name="pgc")
            for ci in range(4):
                st = gn.tile([128, 2, 6], f32, tag="st")
                xv = xt[ci].rearrange("p (n f) -> p n f", f=512)
                for sub in range(2):
                    nc.vector.bn_stats(out=st[:, sub, :], in_=xv[:, sub, :])
                nc.vector.bn_aggr(out=chmv[:, ci, :], in_=st)
                # per-channel [mean, var+mean^2] as f32r for the mask matmul
                nc.vector.tensor_copy(out=s2ch[:, ci, 0:1], in_=chmv[:, ci, 0:1])
                t1 = gn.tile([128, 4], f32, tag="t1")
                nc.vector.tensor_mul(out=t1[:, ci:ci + 1], in0=chmv[:, ci, 0:1],
                                     in1=chmv[:, ci, 0:1])
                nc.vector.tensor_add(out=s2ch[:, ci, 1:2], in0=t1[:, ci:ci + 1],
                                     in1=chmv[:, ci, 1:2])
                # group sums for this tile's 8 groups
                nc.tensor.matmul(pg[0:8, 2 * ci:2 * ci + 2], lhsT=mA,
                                 rhs=s2ch[:, ci, :], start=True, stop=True)
                nc.vector.tensor_scalar_mul(out=mg[:, ci:ci + 1],
                                            in0=pg[0:8, 2 * ci:2 * ci + 1],
                                            scalar1=1.0 / GS)
                nc.vector.tensor_scalar_mul(out=vg[:, ci:ci + 1],
                                            in0=pg[0:8, 2 * ci + 1:2 * ci + 2],
                                            scalar1=1.0 / GS)
                nc.vector.tensor_mul(out=m2[:, ci:ci + 1], in0=mg[:, ci:ci + 1],
                                     in1=mg[:, ci:ci + 1])
                nc.vector.tensor_sub(out=vg[:, ci:ci + 1], in0=vg[:, ci:ci + 1],
                                     in1=m2[:, ci:ci + 1])
                # rstd = 1/sqrt(vg + eps)
                nc.scalar.activation(out=vg[:, ci:ci + 1], in_=vg[:, ci:ci + 1],
                                     func=Act.Sqrt, bias=eps_t[:8], scale=1.0)
                nc.vector.reciprocal(out=vg[:, ci:ci + 1], in_=vg[:, ci:ci + 1])
                nc.vector.tensor_copy(out=gf[:, ci, 0:1], in_=mg[:, ci:ci + 1])
                nc.vector.tensor_copy(out=gf[:, ci, 1:2], in_=vg[:, ci:ci + 1])
                # broadcast group stats back to the tile's 128 channels
                nc.tensor.matmul(pc[:, 2 * ci:2 * ci + 2], lhsT=mB,
                                 rhs=gf[:, ci, :], start=True, stop=True)
                nc.vector.tensor_mul(out=scl[:, ci:ci + 1], in0=gam_t[:, ci:ci + 1],
                                     in1=pc[:, 2 * ci + 1:2 * ci + 2])
                nc.vector.tensor_mul(out=sht[:, ci:ci + 1], in0=scl[:, ci:ci + 1],
                                     in1=pc[:, 2 * ci:2 * ci + 1])
                nc.vector.tensor_sub(out=sht[:, ci:ci + 1], in0=bet_t[:, ci:ci + 1],
                                     in1=sht[:, ci:ci + 1])
                # h = x * scl + sht   (fp8, feeds the DoubleRow matmuls)
                nc.vector.tensor_scalar(out=ht[:, ci, :], in0=xt[ci],
                                        scalar1=scl[:, ci:ci + 1], scalar2=sht[:, ci:ci + 1],
                                        op0=Alu.mult, op1=Alu.add)

            # ---- vT[t, o_v] first (needs only h), so attention can start as
            # soon as the first qkv pair lands ----
            # cols 64:128 of each [t, head] block are all-ones: the AV matmul
            # then emits the softmax denominator Z broadcast across partitions
            # 64:128 of its PSUM tile (matmul cost only depends on N).
            vTa = big.tile([128, 8, 8, 128], f8)  # [t_part, ti, hd, ch+ones]
            nc.gpsimd.memset(vTa[:, :, :, 64:128], 1.0)
            for tp in range(4):
                pv = ps.tile([128, 1024], f32, tag="s", name=f"pv{tp}")
                for half in range(2):
                    ti = 2 * tp + half
                    for g in range(2):
                        nc.tensor.matmul(pv[:, half * 512:(half + 1) * 512],
                                         lhsT=ht[:, 2 * g:2 * g + 2, ti * 128:(ti + 1) * 128],
                                         rhs=wvT_t[:, 2 * g:2 * g + 2, :],
                                         start=(g == 0), stop=(g == 1), perf_mode=DR)
                nc.vector.tensor_add(
                    out=vTa[:, 2 * tp:2 * tp + 2, :, 0:64],
                    in0=pv.rearrange("p (t h c) -> p t h c", t=2, h=8),
                    in1=bass.AP(tensor=bv_b.tensor, offset=bv_b.offset,
                                ap=[list(bv_b.ap[0])] + [[0, 2]] + [[64, 8], [1, 64]]))

            # ---- q,k for one head pair (f32r; scores keep full precision) ----
            qp = [None] * 4
            kpad = [None] * 8

            def qkv_pair(pi):
                qp[pi] = qpp.tile([128, 1024], f32r, tag="qp", name=f"qp{pi}")
                kpad[2 * pi] = kpp.tile([128, 1024], f32r, tag="kp", name=f"kp{2*pi}")
                kpad[2 * pi + 1] = kpp.tile([128, 1024], f32r, tag="kp", name=f"kp{2*pi+1}")
                nc.gpsimd.memset(kpad[2 * pi][64:128, :].bitcast(f32), 0.0)
                nc.gpsimd.memset(kpad[2 * pi + 1][0:64, :].bitcast(f32), 0.0)
                for side in range(2):
                    oi = side * 4 + pi
                    pqk = ps.tile([128, 1024], f32, tag="s", name=f"pqk{oi}")
                    for g in range(2):
                        for ni in range(2):
                            nc.tensor.matmul(
                                pqk[:, ni * 512:(ni + 1) * 512],
                                lhsT=wqkT_t[:, 2 * g:2 * g + 2, oi * 128:(oi + 1) * 128],
                                rhs=ht[:, 2 * g:2 * g + 2, ni * 512:(ni + 1) * 512],
                                start=(g == 0), stop=(g == 1), perf_mode=DR)
                    if side == 0:
                        nc.vector.tensor_scalar_add(out=qp[pi], in0=pqk,
                                                    scalar1=bqk_t[:, oi:oi + 1])
                    else:
                        nc.vector.tensor_scalar_add(
                            out=kpad[2 * pi][0:64, :], in0=pqk[0:64, :],
                            scalar1=bqk_t[0:64, oi:oi + 1])
                        nc.vector.tensor_scalar_add(
                            out=kpad[2 * pi + 1][64:128, :], in0=pqk[64:128, :],
                            scalar1=bqk_t[64:128, oi:oi + 1])

            # ---- attention (one head pair at a time) ----
            at_all = big.tile([128, 4, 1024], f8, tag="at", name="at")
            acc = [accp.tile([128, 1024], f32, tag=f"acc{oi}", name=f"acc{oi}")
                   for oi in range(4)]
            pa_all = [None] * 4

            def attn_core(pi):
                pa = [ps.tile([128, 1024], f32, tag="a", name=f"pa{pi}_{i}") for i in range(2)]
                pa_all[pi] = pa
                es2 = [None, None]
                for si in range(8):
                    u, j = si // 2, si % 2
                    if j == 0:
                        es2 = [esp.tile([128, 2, 1024], f8, tag="es",
                                        name=f"es{pi}_{u}_{k}") for k in range(2)]
                    pss = [ps.tile([128, 1024], f32, tag="s", name=f"pss{pi}_{si}_{i}")
                           for i in range(2)]
                    for half in range(2):
                        for ni in range(2):
                            nc.tensor.matmul(
                                pss[half][:, ni * 512:(ni + 1) * 512],
                                lhsT=kpad[2 * pi + half][:, si * 128:(si + 1) * 128],
                                rhs=qp[pi][:, ni * 512:(ni + 1) * 512],
                                start=True, stop=True)
                    for half in range(2):
                        # es = exp(S - ln16) as fp8; the si-pair shares one es
                        # tile = the two DoubleRow K-tiles of the AV matmul
                        nc.scalar.activation(out=es2[half][:, j, :], in_=pss[half],
                                             func=Act.Exp, bias=nl16_t, scale=1.0)
                    if j == 1:
                        for half in range(2):
                            hd = 2 * pi + half
                            for ni in range(2):
                                nc.tensor.matmul(
                                    pa[half][:, ni * 512:(ni + 1) * 512],
                                    lhsT=vTa[:, 2 * u:2 * u + 2, hd, :],
                                    rhs=es2[half][:, :, ni * 512:(ni + 1) * 512],
                                    start=(u == 0), stop=(u == 3), perf_mode=DR)

            def divide(pi, tail=False):
                """at[pi] = AV / Z.  Z arrives pre-broadcast on partitions
                64:128 of the AV PSUM tile (ones-columns in vTa).
                Exact DVE reciprocal costs ~4.7ns/free-elem, so the fast path
                reshapes Z to [128, 8] via a DRAM round trip (off the critical
                path; an early copy releases the PSUM banks).  The tail pair
                instead computes 1/Z = Exp(-Ln(Z)) on the then-idle ACT engine
                (ln+exp share an activation table), split by t-halves so the
                tail proj can chase it."""
                pa = pa_all[pi]
                if tail:
                    for ni in range(2):
                        sl = slice(ni * 512, (ni + 1) * 512)
                        for half in range(2):
                            lz = zp.tile([64, 1024], f32, tag="lz")
                            nc.scalar.activation(out=lz[:, sl], in_=pa[half][64:128, sl],
                                                 func=Act.Ln)
                            rzb = zp.tile([64, 1024], f32, tag="rz")
                            nc.scalar.activation(out=rzb[:, sl], in_=lz[:, sl],
                                                 func=Act.Exp, scale=-1.0)
                            nc.vector.tensor_tensor(
                                out=at_all[half * 64:half * 64 + 64, pi, sl],
                                in0=pa[half][0:64, sl], in1=rzb[:, sl], op=Alu.mult)
                    return
                aus = []
                for half in range(2):
                    au = zp.tile([65, 1024], f32, tag="au")
                    nc.vector.tensor_copy(out=au, in_=pa[half][0:65, :])
                    aus.append(au)
                for half in range(2):
                    eng = nc.sync if half == 0 else nc.gpsimd
                    zd = dram.tile([1, 1024], f32, tag="zd")
                    eng.dma_start(out=zd, in_=aus[half][64:65, :])
                    zq = zp.tile([128, 8], f32, tag="zq")
                    eng.dma_start(out=zq, in_=bass.AP(
                        tensor=zd.tensor, offset=zd.offset, ap=[[8, 128], [1, 8]]))
                    nc.vector.reciprocal(out=zq, in_=zq)
                    zd2 = dram.tile([1, 1024], f32, tag="zd2")
                    eng.dma_start(
                        out=bass.AP(tensor=zd2.tensor, offset=zd2.offset,
                                    ap=[[8, 128], [1, 8]]), in_=zq)
                    zb = zp.tile([64, 1024], f32, tag="zb")
                    eng.dma_start(out=zb, in_=bass.AP(
                        tensor=zd2.tensor, offset=zd2.offset, ap=[[0, 64], [1, 1024]]))
                    nc.vector.tensor_tensor(
                        out=at_all[half * 64:half * 64 + 64, pi, :],
                        in0=aus[half][0:64, :], in1=zb, op=Alu.mult)

            def proj01():
                """pairs {0,1}: one DoubleRow pass per (oi, ni) + bias/residual."""
                for oi in range(4):
                    pp = ps.tile([128, 1024], f32, tag="s", name=f"pp01_{oi}")
                    for ni in range(2):
                        nc.tensor.matmul(pp[:, ni * 512:(ni + 1) * 512],
                                         lhsT=wpT_t[:, 0:2, oi * 128:(oi + 1) * 128],
                                         rhs=at_all[:, 0:2, ni * 512:(ni + 1) * 512],
                                         start=True, stop=True, perf_mode=DR)
                    nc.vector.scalar_tensor_tensor(
                        out=acc[oi], in0=pp, scalar=bp_t[:, oi:oi + 1],
                        in1=xt[oi], op0=Alu.add, op1=Alu.add)

            def proj_tail():
                """pairs {2,3}: per output block one DoubleRow pass per ni ->
                add -> DMA out, pipelined behind the t-half-split divide(3)."""
                oeng = [nc.sync, nc.scalar, nc.gpsimd, nc.sync]
                outr = outd.rearrange("(ci p) t -> p ci t", p=128)
                for oi in range(4):
                    pp = ps.tile([128, 1024], f32, tag=("a" if oi >= 2 else "s"),
                                 name=f"ppt_{oi}")
                    for ni in range(2):
                        nc.tensor.matmul(pp[:, ni * 512:(ni + 1) * 512],
                                         lhsT=wpT_t[:, 2:4, oi * 128:(oi + 1) * 128],
                                         rhs=at_all[:, 2:4, ni * 512:(ni + 1) * 512],
                                         start=True, stop=True, perf_mode=DR)
                    nc.vector.tensor_add(out=acc[oi], in0=acc[oi], in1=pp)
                    oeng[oi].dma_start(out=outr[:, oi, :], in_=acc[oi])

            qkv_pair(0)
            attn_core(0)
            divide(0)
            qkv_pair(1)
            attn_core(1)
            divide(1)
            qkv_pair(2)
            attn_core(2)
            divide(2)
            qkv_pair(3)
            proj01()
            attn_core(3)
            divide(3, tail=True)
            proj_tail()

    _split_multi_waits(nc)
    return nc


def _prep_host(x, gamma, beta, w_qkv, b_qkv, w_proj, b_proj):
    """Host-side weight permutation/pre-scaling + per-core input maps."""
    import ml_dtypes
    x = np.ascontiguousarray(x, dtype=np.float32).reshape(B, C, T)
    scale2 = 1.0 / np.sqrt(CH)  # folded into q (exact: 0.125 is a power of two)

    w = np.asarray(w_qkv, dtype=np.float32).reshape(H, 3, CH, C)
    bq = np.asarray(b_qkv, dtype=np.float32).reshape(H, 3, CH)
    wq = w[:, 0] * scale2          # [hd, 64, c]
    wk = w[:, 1]
    wv = w[:, 2]
    # o-block order: 4 q-blocks (one per head pair: [q_{2i}; q_{2i+1}]), 4 k-blocks
    qcols = wq.reshape(4, 128, C).transpose(2, 0, 1)            # [c, pi, 128]
    kcols = wk.reshape(4, 128, C).transpose(2, 0, 1)
    wqkT_host = np.ascontiguousarray(
        np.concatenate([qcols, kcols], axis=1).reshape(C, 1024)
    ).astype(ml_dtypes.float8_e4m3)
    bqk_host = np.ascontiguousarray(np.concatenate(
        [(bq[:, 0] * scale2).reshape(4, 128), bq[:, 1].reshape(4, 128)], axis=0
    ).reshape(1024))
    wvT_host = np.ascontiguousarray(
        wv.transpose(2, 0, 1).reshape(C, C)).astype(ml_dtypes.float8_e4m3)
    bv_host = np.ascontiguousarray(bq[:, 2].reshape(C))
    wpT_host = np.ascontiguousarray(
        np.asarray(w_proj, dtype=np.float32).T).astype(ml_dtypes.float8_e4m3)
    maskA_host = np.zeros((128, 8), np.float32)
    for p in range(128):
        maskA_host[p, p // 16] = 1.0
    maskB_host = np.ascontiguousarray(maskA_host.T)

    common = {
        "wqkT": wqkT_host, "wvT8": wvT_host, "wpT8": wpT_host,
        "bqk": bqk_host, "bv": bv_host,
        "bp": np.ascontiguousarray(np.asarray(b_proj, np.float32)),
        "gam": np.ascontiguousarray(np.asarray(gamma, np.float32)),
        "bet": np.ascontiguousarray(np.asarray(beta, np.float32)),
        "maskA": maskA_host, "maskB": maskB_host,
    }
    return [dict(common, xin=np.ascontiguousarray(x[b])) for b in range(B)]


def kernel(x, gamma, beta, w_qkv, b_qkv, w_proj, b_proj):
    from concourse.bass_utils import run_bass_kernel_spmd

    if "nc" not in _CACHE:
        _CACHE["nc"] = _build_nc()
    nc = _CACHE["nc"]

    in_maps = _prep_host(x, gamma, beta, w_qkv, b_qkv, w_proj, b_proj)
    kwargs = {}
    if TRACE:
        _install_ntff_hook()
        kwargs["trace"] = True
    res = run_bass_kernel_spmd(nc, in_maps, core_ids=list(range(NCORES)), **kwargs)
    if TRACE:
        _CACHE["last_result"] = res
    out = np.stack([r["outd"] for r in res.results], axis=0)
    return out.reshape(B, C, HW, HW)


# revision 27
# speedup vs baseline: 1.2388x; 1.0558x over previous
"""Trainium2 Bass kernel for nn_AttentionBlock (GroupNorm + MHA + proj + residual).

Sharding: data-parallel over batch; 8 batches -> 8 NeuronCores, one batch each.

Per-core layout (c=512 channels, t=1024 spatial, H=8 heads, ch=64):
  - x kept as [c-on-partitions, t] (4 tiles of [128, 1024], fp32)
  - GroupNorm pipelined per 128-channel tile (each tile holds 8 complete
    groups), group reduction via two tiny mask matmuls on the PE
  - h, v/vT, softmax weights (es), attention output (at) and the qkv/proj
    weights are fp8 e4m3; qkv/v/AV/proj matmuls run in DoubleRow perf mode
    (2 stacked K-tiles per pass -> half the matmul count).  Scores stay f32r
    (contraction is only 64 deep -> DoubleRow gains nothing, keeps precision).
  - q,k per head PAIR: q paired [q_A; q_B] on 128 partitions; per-head k
    zero-padded to 128 partitions (f32r, cost only depends on N)
  - scores computed transposed: S^T[s, t] = k^T q; exp(S^T - ln16) on ACT
    writes fp8 es directly (the 1/16 keeps es in e4m3 range and cancels in the
    softmax normalization); si-pairs share one es tile = the two DoubleRow
    K-tiles of the AV matmul
  - vT carries 64 all-ones columns -> the AV matmul emits the softmax
    denominator Z broadcast on partitions 64:128 of its PSUM tile
  - Z normalization: pairs 0-2 reshape Z to [128, 8] via a DRAM round trip for
    a cheap exact DVE reciprocal (off the critical path; an early copy releases
    the PSUM banks).  Pair 3 computes 1/Z = Exp(-Ln(Z)) on the then-idle ACT
    engine (ln+exp share an activation table), split by t-halves so the tail
    proj can chase it.
  - proj: pairs {0,1} and {2,3} are DoubleRow K-tile pairs; {0,1} runs between
    attn cores 2 and 3, {2,3} in the tail, each + bias/residual on DVE.
  - input DMAs: x tiles split across the 3 DMA-capable queues ahead of all
    weights; the vector queue stays DMA-free so GroupNorm starts ASAP.
"""

import numpy as np

B, C, HW, T = 8, 512, 32, 1024
H, CH = 8, 64
G, GS = 32, 16
EPS = 1e-5
NCORES = 8
LOG16 = float(np.log(16.0))

_CACHE = {}
TRACE = False  # test harness can set kernel.TRACE = True to get a profile


def _install_ntff_hook():
    import sys, types
    if 'antenv.axon_hooks' in sys.modules:
        return
    mod = types.ModuleType('antenv.axon_hooks')
    state = {'hook': None}
    mod.set_axon_ntff_profile_hook = lambda h: state.__setitem__('hook', h)
    mod.get_axon_ntff_profile_hook = lambda: state['hook']
    sys.modules['antenv.axon_hooks'] = mod
    import antenv
    antenv.axon_hooks = mod
    try:
        from trn_agent_boot.trn_boot import _ntff_profile_via_ctypes
        mod.set_axon_ntff_profile_hook(_ntff_profile_via_ctypes('/opt/axon/libaxon_pjrt.so'))
    except Exception:
        pass


def _split_multi_waits(nc, max_waits=1):
    """This container's walrus supports only one sync wait per instruction; move
    extra waits onto same-engine no-ops inserted just before the instruction."""
    import concourse.mybir as mybir
    for f in nc.m.functions:
        for bb in f.blocks:
            insts = bb.instructions
            out = []
            changed = False
            for inst in insts:
                si = inst.sync_info
                waits = list(si.on_wait) if si is not None and si.on_wait else []
                if len(waits) > max_waits:
                    changed = True
                    for j, w in enumerate(waits[:-max_waits]):
                        out.append(mybir.InstNoOp(
                            name=f"{inst.name}-ws{j}",
                            sync_info=mybir.SyncInfo(on_wait=[w], on_update=[]),
                            bass_nofuse=True,
                            engine=inst.engine,
                        ))
                    inst.sync_info = mybir.SyncInfo(
                        on_wait=waits[-max_waits:],
                        on_update=list(si.on_update) if si.on_update else [],
                    )
                out.append(inst)
            if changed:
                bb.instructions = out


def _build_nc():
    import concourse.bass as bass
    import concourse.tile as tile
    import concourse.mybir as mybir

    f32 = mybir.dt.float32
    f32r = mybir.dt.float32r
    f8 = mybir.dt.float8e4
    Alu = mybir.AluOpType
    Act = mybir.ActivationFunctionType
    DR = mybir.MatmulPerfMode.DoubleRow

    nc = bass.Bass()

    xin = nc.dram_tensor("xin", [C, T], f32, kind="ExternalInput")
    wqkT = nc.dram_tensor("wqkT", [C, 1024], f8, kind="ExternalInput")
    wvT8 = nc.dram_tensor("wvT8", [C, C], f8, kind="ExternalInput")
    wpT8 = nc.dram_tensor("wpT8", [C, C], f8, kind="ExternalInput")
    bqk = nc.dram_tensor("bqk", [1024], f32, kind="ExternalInput")
    bv = nc.dram_tensor("bv", [C], f32, kind="ExternalInput")
    bp = nc.dram_tensor("bp", [C], f32, kind="ExternalInput")
    gam = nc.dram_tensor("gam", [C], f32, kind="ExternalInput")
    bet = nc.dram_tensor("bet", [C], f32, kind="ExternalInput")
    maskA = nc.dram_tensor("maskA", [128, 8], f32r, kind="ExternalInput")
    maskB = nc.dram_tensor("maskB", [8, 128], f32r, kind="ExternalInput")
    outd = nc.dram_tensor("outd", [C, T], f32, kind="ExternalOutput")

    with tile.TileContext(nc) as tc:
        with tc.tile_pool(name="const", bufs=1) as const, \
             tc.tile_pool(name="big", bufs=1) as big, \
             tc.tile_pool(name="qpp", bufs=2) as qpp, \
             tc.tile_pool(name="kpp", bufs=4) as kpp, \
             tc.tile_pool(name="esp", bufs=6) as esp, \
             tc.tile_pool(name="accp", bufs=1) as accp, \
             tc.tile_pool(name="zp", bufs=2) as zp, \
             tc.tile_pool(name="gn", bufs=2) as gn, \
             tc.tile_pool(name="ps", bufs=2, space="PSUM") as ps, \
             tc.tile_pool(name="dram", bufs=2, space="DRAM") as dram:

            # ---- input DMAs.  x tiles first so GroupNorm can start ~5us in;
            # weights follow on the same queues.  The vector queue issues no
            # DMAs (it runs GroupNorm + evacuations). ----
            xt = [big.tile([128, 1024], f32, tag=f"x{ci}", name=f"x{ci}") for ci in range(4)]
            xr = xin.rearrange("(ci p) t -> p ci t", p=128)
            nc.sync.dma_start(out=xt[0], in_=xr[:, 0, :])
            nc.scalar.dma_start(out=xt[1], in_=xr[:, 1, :])
            nc.gpsimd.dma_start(out=xt[2], in_=xr[:, 2, :])
            nc.sync.dma_start(out=xt[3][:, 0:512], in_=xr[:, 3, 0:512])
            nc.scalar.dma_start(out=xt[3][:, 512:1024], in_=xr[:, 3, 512:1024])

            # tiny GN constants right behind x on sync/scalar (land ~x-time)
            gam_t = const.tile([128, 4], f32)
            nc.sync.dma_start(out=gam_t, in_=gam.rearrange("(ci p) -> p ci", p=128))
            bet_t = const.tile([128, 4], f32)
            nc.sync.dma_start(out=bet_t, in_=bet.rearrange("(ci p) -> p ci", p=128))
            mA = const.tile([128, 8], f32r)
            nc.scalar.dma_start(out=mA, in_=maskA[:, :])
            mB = const.tile([8, 128], f32r)
            nc.scalar.dma_start(out=mB, in_=maskB[:, :])
            eps_t = const.tile([128, 1], f32)
            nc.vector.memset(eps_t, EPS)
            nl16_t = const.tile([128, 1], f32)
            nc.vector.memset(nl16_t, -LOG16)

            # v weights next (v runs first after GN)
            wvT_t = const.tile([128, 4, 512], f8)
            wvr = wvT8.rearrange("(ci p) o -> p ci o", p=128)
            nc.sync.dma_start(out=wvT_t[:, 0:2, :], in_=wvr[:, 0:2, :])
            nc.scalar.dma_start(out=wvT_t[:, 2:4, :], in_=wvr[:, 2:4, :])
            # qk weights
            wqkT_t = const.tile([128, 4, 1024], f8)
            wqr = wqkT.rearrange("(ci p) o -> p ci o", p=128)
            weng = [nc.sync, nc.scalar, nc.gpsimd, nc.gpsimd]
            for ci in range(4):
                weng[ci].dma_start(out=wqkT_t[:, ci, :], in_=wqr[:, ci, :])
            # small biases
            bqk_t = const.tile([128, 8], f32)
            nc.gpsimd.dma_start(out=bqk_t, in_=bqk.rearrange("(oi p) -> p oi", p=128))
            bv_b = const.tile([128, 512], f32)
            nc.gpsimd.dma_start(out=bv_b, in_=bass.AP(
                tensor=bv, offset=0, ap=[[0, 128], [1, 512]]))
            bp_t = const.tile([128, 4], f32)
            nc.sync.dma_start(out=bp_t, in_=bp.rearrange("(ci p) -> p ci", p=128))
            # proj weights last (needed latest)
            wpT_t = const.tile([128, 4, 512], f8)
            wpr = wpT8.rearrange("(ci p) o -> p ci o", p=128)
            nc.sync.dma_start(out=wpT_t[:, 0:2, :], in_=wpr[:, 0:2, :])
            nc.scalar.dma_start(out=wpT_t[:, 2:4, :], in_=wpr[:, 2:4, :])

            # ---- GroupNorm, pipelined per 128-channel tile ----
            # each ci tile holds 8 complete groups (16 channels each), so the
            # whole stats -> group-reduce -> normalize chain runs per ci as
            # its x DMA lands, overlapping the remaining x/weight DMAs.
            # The tiny mask matmuls use PSUM tag "a" (idle until attention).
            ht = big.tile([128, 4, 1024], f8, tag="h", name="h")
            chmv = gn.tile([128, 4, 2], f32)
            s2ch = gn.tile([128, 4, 2], f32r)
            gf = gn.tile([8, 4, 2], f32r)
            mg = gn.tile([8, 4], f32)
            vg = gn.tile([8, 4], f32)
            m2 = gn.tile([8, 4], f32)
            scl = gn.tile([128, 4], f32)
            sht = gn.tile([128, 4], f32)
            pg = ps.tile([128, 1024], f32, tag="a", name="pgn")
            pc = ps.tile([128, 1024], f32, tag="a", name="pgc")
            for ci in range(4):
                st = gn.tile([128, 2, 6], f32, tag="st")
                xv = xt[ci].rearrange("p (n f) -> p n f", f=512)
                for sub in range(2):
                    nc.vector.bn_stats(out=st[:, sub, :], in_=xv[:, sub, :])
                nc.vector.bn_aggr(out=chmv[:, ci, :], in_=st)
                # per-channel [mean, var+mean^2] as f32r for the mask matmul
                nc.vector.tensor_copy(out=s2ch[:, ci, 0:1], in_=chmv[:, ci, 0:1])
                t1 = gn.tile([128, 4], f32, tag="t1")
                nc.vector.tensor_mul(out=t1[:, ci:ci + 1], in0=chmv[:, ci, 0:1],
                                     in1=chmv[:, ci, 0:1])
                nc.vector.tensor_add(out=s2ch[:, ci, 1:2], in0=t1[:, ci:ci + 1],
                                     in1=chmv[:, ci, 1:2])
                # group sums for this tile's 8 groups
                nc.tensor.matmul(pg[0:8, 2 * ci:2 * ci + 2], lhsT=mA,
                                 rhs=s2ch[:, ci, :], start=True, stop=True)
                nc.vector.tensor_scalar_mul(out=mg[:, ci:ci + 1],
                                            in0=pg[0:8, 2 * ci:2 * ci + 1],
                                            scalar1=1.0 / GS)
                nc.vector.tensor_scalar_mul(out=vg[:, ci:ci + 1],
                                            in0=pg[0:8, 2 * ci + 1:2 * ci + 2],
                                            scalar1=1.0 / GS)
                nc.vector.tensor_mul(out=m2[:, ci:ci + 1], in0=mg[:, ci:ci + 1],
                                     in1=mg[:, ci:ci + 1])
                nc.vector.tensor_sub(out=vg[:, ci:ci + 1], in0=vg[:, ci:ci + 1],
                                     in1=m2[:, ci:ci + 1])
                # rstd = 1/sqrt(vg + eps)
                nc.scalar.activation(out=vg[:, ci:ci + 1], in_=vg[:, ci:ci + 1],
                                     func=Act.Sqrt, bias=eps_t[:8], scale=1.0)
                nc.vector.reciprocal(out=vg[:, ci:ci + 1], in_=vg[:, ci:ci + 1])
                nc.vector.tensor_copy(out=gf[:, ci, 0:1], in_=mg[:, ci:ci + 1])
                nc.vector.tensor_copy(out=gf[:, ci, 1:2], in_=vg[:, ci:ci + 1])
                # broadcast group stats back to the tile's 128 channels
                nc.tensor.matmul(pc[:, 2 * ci:2 * ci + 2], lhsT=mB,
                                 rhs=gf[:, ci, :], start=True, stop=True)
                nc.vector.tensor_mul(out=scl[:, ci:ci + 1], in0=gam_t[:, ci:ci + 1],
                                     in1=pc[:, 2 * ci + 1:2 * ci + 2])
                nc.vector.tensor_mul(out=sht[:, ci:ci + 1], in0=scl[:, ci:ci + 1],
                                     in1=pc[:, 2 * ci:2 * ci + 1])
                nc.vector.tensor_sub(out=sht[:, ci:ci + 1], in0=bet_t[:, ci:ci + 1],
                                     in1=sht[:, ci:ci + 1])
                # h = x * scl + sht   (fp8, feeds the DoubleRow matmuls)
                nc.vector.tensor_scalar(out=ht[:, ci, :], in0=xt[ci],
                                        scalar1=scl[:, ci:ci + 1], scalar2=sht[:, ci:ci + 1],
                                        op0=Alu.mult, op1=Alu.add)

            # ---- vT[t, o_v] first (needs only h), so attention can start as
            # soon as the first qkv pair lands ----
            # cols 64:128 of each [t, head] block are all-ones: the AV matmul
            # then emits the softmax denominator Z broadcast across partitions
            # 64:128 of its PSUM tile (matmul cost only depends on N).
            vTa = big.tile([128, 8, 8, 128], f8)  # [t_part, ti, hd, ch+ones]
            nc.gpsimd.memset(vTa[:, :, :, 64:128], 1.0)
            for tp in range(4):
                pv = ps.tile([128, 1024], f32, tag="s", name=f"pv{tp}")
                for half in range(2):
                    ti = 2 * tp + half
                    for g in range(2):
                        nc.tensor.matmul(pv[:, half * 512:(half + 1) * 512],
                                         lhsT=ht[:, 2 * g:2 * g + 2, ti * 128:(ti + 1) * 128],
                                         rhs=wvT_t[:, 2 * g:2 * g + 2, :],
                                         start=(g == 0), stop=(g == 1), perf_mode=DR)
                nc.vector.tensor_add(
                    out=vTa[:, 2 * tp:2 * tp + 2, :, 0:64],
                    in0=pv.rearrange("p (t h c) -> p t h c", t=2, h=8),
                    in1=bass.AP(tensor=bv_b.tensor, offset=bv_b.offset,
                                ap=[list(bv_b.ap[0])] + [[0, 2]] + [[64, 8], [1, 64]]))

            # ---- q,k for one head pair (f32r; scores keep full precision) ----
            qp = [None] * 4
            kpad = [None] * 8

            def qkv_side(pi, side):
                oi = side * 4 + pi
                pqk = ps.tile([128, 1024], f32, tag="s", name=f"pqk{oi}")
                for g in range(2):
                    for ni in range(2):
                        nc.tensor.matmul(
                            pqk[:, ni * 512:(ni + 1) * 512],
                            lhsT=wqkT_t[:, 2 * g:2 * g + 2, oi * 128:(oi + 1) * 128],
                            rhs=ht[:, 2 * g:2 * g + 2, ni * 512:(ni + 1) * 512],
                            start=(g == 0), stop=(g == 1), perf_mode=DR)
                if side == 0:
                    nc.vector.tensor_scalar_add(out=qp[pi], in0=pqk,
                                                scalar1=bqk_t[:, oi:oi + 1])
                else:
                    nc.vector.tensor_scalar_add(
                        out=kpad[2 * pi][0:64, :], in0=pqk[0:64, :],
                        scalar1=bqk_t[0:64, oi:oi + 1])
                    nc.vector.tensor_scalar_add(
                        out=kpad[2 * pi + 1][64:128, :], in0=pqk[64:128, :],
                        scalar1=bqk_t[64:128, oi:oi + 1])

            def qkv_alloc(pi):
                qp[pi] = qpp.tile([128, 1024], f32r, tag="qp", name=f"qp{pi}")
                kpad[2 * pi] = kpp.tile([128, 1024], f32r, tag="kp", name=f"kp{2*pi}")
                kpad[2 * pi + 1] = kpp.tile([128, 1024], f32r, tag="kp", name=f"kp{2*pi+1}")
                nc.gpsimd.memset(kpad[2 * pi][64:128, :].bitcast(f32), 0.0)
                nc.gpsimd.memset(kpad[2 * pi + 1][0:64, :].bitcast(f32), 0.0)

            def qkv_pair(pi):
                qkv_alloc(pi)
                qkv_side(pi, 0)
                qkv_side(pi, 1)

            # ---- attention (one head pair at a time) ----
            at_all = big.tile([128, 4, 1024], f8, tag="at", name="at")
            acc = [accp.tile([128, 1024], f32, tag=f"acc{oi}", name=f"acc{oi}")
                   for oi in range(4)]
            pa_all = [None] * 4

            def attn_core(pi, prefetch=()):
                """prefetch: {si: thunk} emitted after that si step's matmuls —
                used to overlap the next pair's qkv work with this pair's
                exp-bound pipeline (all prefetch deps must already be ready
                so the FIFO PSUM slot grants can't stall the score allocs)."""
                prefetch = dict(prefetch)
                pa = [ps.tile([128, 1024], f32, tag="a", name=f"pa{pi}_{i}") for i in range(2)]
                pa_all[pi] = pa
                es2 = [None, None]
                for si in range(8):
                    u, j = si // 2, si % 2
                    if j == 0:
                        es2 = [esp.tile([128, 2, 1024], f8, tag="es",
                                        name=f"es{pi}_{u}_{k}") for k in range(2)]
                    if si in prefetch:
                        prefetch[si]()
                    pss = [ps.tile([128, 1024], f32, tag="s", name=f"pss{pi}_{si}_{i}")
                           for i in range(2)]
                    for half in range(2):
                        for ni in range(2):
                            nc.tensor.matmul(
                                pss[half][:, ni * 512:(ni + 1) * 512],
                                lhsT=kpad[2 * pi + half][:, si * 128:(si + 1) * 128],
                                rhs=qp[pi][:, ni * 512:(ni + 1) * 512],
                                start=True, stop=True)
                    for half in range(2):
                        # es = exp(S - ln16) as fp8; the si-pair shares one es
                        # tile = the two DoubleRow K-tiles of the AV matmul
                        nc.scalar.activation(out=es2[half][:, j, :], in_=pss[half],
                                             func=Act.Exp, bias=nl16_t, scale=1.0)
                    if j == 1:
                        for half in range(2):
                            hd = 2 * pi + half
                            for ni in range(2):
                                nc.tensor.matmul(
                                    pa[half][:, ni * 512:(ni + 1) * 512],
                                    lhsT=vTa[:, 2 * u:2 * u + 2, hd, :],
                                    rhs=es2[half][:, :, ni * 512:(ni + 1) * 512],
                                    start=(u == 0), stop=(u == 3), perf_mode=DR)

            def divide(pi, tail=False):
                """at[pi] = AV / Z.  Z arrives pre-broadcast on partitions
                64:128 of the AV PSUM tile (ones-columns in vTa).
                Exact DVE reciprocal costs ~4.7ns/free-elem, so the fast path
                reshapes Z to [128, 8] via a DRAM round trip (off the critical
                path; an early copy releases the PSUM banks).  The tail pair
                instead computes 1/Z = Exp(-Ln(Z)) on the then-idle ACT engine
                (ln+exp share an activation table), split by t-halves so the
                tail proj can chase it."""
                pa = pa_all[pi]
                if tail:
                    for ni in range(2):
                        sl = slice(ni * 512, (ni + 1) * 512)
                        for half in range(2):
                            lz = zp.tile([64, 1024], f32, tag="lz")
                            nc.scalar.activation(out=lz[:, sl], in_=pa[half][64:128, sl],
                                                 func=Act.Ln)
                            rzb = zp.tile([64, 1024], f32, tag="rz")
                            nc.scalar.activation(out=rzb[:, sl], in_=lz[:, sl],
                                                 func=Act.Exp, scale=-1.0)
                            nc.vector.tensor_tensor(
                                out=at_all[half * 64:half * 64 + 64, pi, sl],
                                in0=pa[half][0:64, sl], in1=rzb[:, sl], op=Alu.mult)
                    return
                aus = []
                for half in range(2):
                    au = zp.tile([65, 1024], f32, tag="au")
                    nc.vector.tensor_copy(out=au, in_=pa[half][0:65, :])
                    aus.append(au)
                for half in range(2):
                    eng = nc.sync if half == 0 else nc.gpsimd
                    zd = dram.tile([1, 1024], f32, tag="zd")
                    eng.dma_start(out=zd, in_=aus[half][64:65, :])
                    zq = zp.tile([128, 8], f32, tag="zq")
                    eng.dma_start(out=zq, in_=bass.AP(
                        tensor=zd.tensor, offset=zd.offset, ap=[[8, 128], [1, 8]]))
                    nc.vector.reciprocal(out=zq, in_=zq)
                    zd2 = dram.tile([1, 1024], f32, tag="zd2")
                    eng.dma_start(
                        out=bass.AP(tensor=zd2.tensor, offset=zd2.offset,
                                    ap=[[8, 128], [1, 8]]), in_=zq)
                    zb = zp.tile([64, 1024], f32, tag="zb")
                    eng.dma_start(out=zb, in_=bass.AP(
                        tensor=zd2.tensor, offset=zd2.offset, ap=[[0, 64], [1, 1024]]))
                    nc.vector.tensor_tensor(
                        out=at_all[half * 64:half * 64 + 64, pi, :],
                        in0=aus[half][0:64, :], in1=zb, op=Alu.mult)

            def proj01():
                """pairs {0,1}: one DoubleRow pass per (oi, ni) + bias/residual."""
                for oi in range(4):
                    pp = ps.tile([128, 1024], f32, tag="s", name=f"pp01_{oi}")
                    for ni in range(2):
                        nc.tensor.matmul(pp[:, ni * 512:(ni + 1) * 512],
                                         lhsT=wpT_t[:, 0:2, oi * 128:(oi + 1) * 128],
                                         rhs=at_all[:, 0:2, ni * 512:(ni + 1) * 512],
                                         start=True, stop=True, perf_mode=DR)
                    nc.vector.scalar_tensor_tensor(
                        out=acc[oi], in0=pp, scalar=bp_t[:, oi:oi + 1],
                        in1=xt[oi], op0=Alu.add, op1=Alu.add)

            def proj_tail():
                """pairs {2,3}: per output block one DoubleRow pass per ni ->
                add -> DMA out, pipelined behind the t-half-split divide(3)."""
                oeng = [nc.sync, nc.scalar, nc.gpsimd, nc.sync]
                outr = outd.rearrange("(ci p) t -> p ci t", p=128)
                for oi in range(4):
                    pp = ps.tile([128, 1024], f32, tag=("a" if oi >= 2 else "s"),
                                 name=f"ppt_{oi}")
                    for ni in range(2):
                        nc.tensor.matmul(pp[:, ni * 512:(ni + 1) * 512],
                                         lhsT=wpT_t[:, 2:4, oi * 128:(oi + 1) * 128],
                                         rhs=at_all[:, 2:4, ni * 512:(ni + 1) * 512],
                                         start=True, stop=True, perf_mode=DR)
                    nc.vector.tensor_add(out=acc[oi], in0=acc[oi], in1=pp)
                    oeng[oi].dma_start(out=outr[:, oi, :], in_=acc[oi])

            qkv_pair(0)
            qkv_alloc(1)
            attn_core(0, prefetch={3: lambda: qkv_side(1, 0),
                                   5: lambda: qkv_side(1, 1)})
            divide(0)
            qkv_alloc(2)
            attn_core(1, prefetch={3: lambda: qkv_side(2, 0),
                                   5: lambda: qkv_side(2, 1)})
            divide(1)
            qkv_alloc(3)
            attn_core(2, prefetch={3: lambda: qkv_side(3, 0),
                                   5: lambda: qkv_side(3, 1)})
            divide(2)
            proj01()
            attn_core(3)
            divide(3, tail=True)
            proj_tail()

    _split_multi_waits(nc)
    return nc


def _prep_host(x, gamma, beta, w_qkv, b_qkv, w_proj, b_proj):
    """Host-side weight permutation/pre-scaling + per-core input maps."""
    import ml_dtypes
    x = np.ascontiguousarray(x, dtype=np.float32).reshape(B, C, T)
    scale2 = 1.0 / np.sqrt(CH)  # folded into q (exact: 0.125 is a power of two)

    w = np.asarray(w_qkv, dtype=np.float32).reshape(H, 3, CH, C)
    bq = np.asarray(b_qkv, dtype=np.float32).reshape(H, 3, CH)
    wq = w[:, 0] * scale2          # [hd, 64, c]
    wk = w[:, 1]
    wv = w[:, 2]
    # o-block order: 4 q-blocks (one per head pair: [q_{2i}; q_{2i+1}]), 4 k-blocks
    qcols = wq.reshape(4, 128, C).transpose(2, 0, 1)            # [c, pi, 128]
    kcols = wk.reshape(4, 128, C).transpose(2, 0, 1)
    wqkT_host = np.ascontiguousarray(
        np.concatenate([qcols, kcols], axis=1).reshape(C, 1024)
    ).astype(ml_dtypes.float8_e4m3)
    bqk_host = np.ascontiguousarray(np.concatenate(
        [(bq[:, 0] * scale2).reshape(4, 128), bq[:, 1].reshape(4, 128)], axis=0
    ).reshape(1024))
    wvT_host = np.ascontiguousarray(
        wv.transpose(2, 0, 1).reshape(C, C)).astype(ml_dtypes.float8_e4m3)
    bv_host = np.ascontiguousarray(bq[:, 2].reshape(C))
    wpT_host = np.ascontiguousarray(
        np.asarray(w_proj, dtype=np.float32).T).astype(ml_dtypes.float8_e4m3)
    maskA_host = np.zeros((128, 8), np.float32)
    for p in range(128):
        maskA_host[p, p // 16] = 1.0
    maskB_host = np.ascontiguousarray(maskA_host.T)

    common = {
        "wqkT": wqkT_host, "wvT8": wvT_host, "wpT8": wpT_host,
        "bqk": bqk_host, "bv": bv_host,
        "bp": np.ascontiguousarray(np.asarray(b_proj, np.float32)),
        "gam": np.ascontiguousarray(np.asarray(gamma, np.float32)),
        "bet": np.ascontiguousarray(np.asarray(beta, np.float32)),
        "maskA": maskA_host, "maskB": maskB_host,
    }
    return [dict(common, xin=np.ascontiguousarray(x[b])) for b in range(B)]


def kernel(x, gamma, beta, w_qkv, b_qkv, w_proj, b_proj):
    from concourse.bass_utils import run_bass_kernel_spmd

    if "nc" not in _CACHE:
        _CACHE["nc"] = _build_nc()
    nc = _CACHE["nc"]

    in_maps = _prep_host(x, gamma, beta, w_qkv, b_qkv, w_proj, b_proj)
    kwargs = {}
    if TRACE:
        _install_ntff_hook()
        kwargs["trace"] = True
    res = run_bass_kernel_spmd(nc, in_maps, core_ids=list(range(NCORES)), **kwargs)
    if TRACE:
        _CACHE["last_result"] = res
    out = np.stack([r["outd"] for r in res.results], axis=0)
    return out.reshape(B, C, HW, HW)


# revision 36
# speedup vs baseline: 1.2656x; 1.0216x over previous
"""Trainium2 Bass kernel for nn_AttentionBlock (GroupNorm + MHA + proj + residual).

Sharding: data-parallel over batch; 8 batches -> 8 NeuronCores, one batch each.

Per-core layout (c=512 channels, t=1024 spatial, H=8 heads, ch=64):
  - x kept as [c-on-partitions, t] (4 tiles of [128, 1024], fp32)
  - GroupNorm pipelined per 128-channel tile (each tile holds 8 complete
    groups), group reduction via two tiny mask matmuls on the PE
  - h, v/vT, softmax weights (es), attention output (at) and the qkv/proj
    weights are fp8 e4m3; qkv/v/AV/proj matmuls run in DoubleRow perf mode
    (2 stacked K-tiles per pass -> half the matmul count).  Scores stay f32r
    (contraction is only 64 deep -> DoubleRow gains nothing, keeps precision).
  - q,k per head PAIR: q paired [q_A; q_B] on 128 partitions; per-head k
    zero-padded to 128 partitions (f32r, cost only depends on N)
  - scores computed transposed: S^T[s, t] = k^T q; exp(S^T - ln16) on ACT
    writes fp8 es directly (the 1/16 keeps es in e4m3 range and cancels in the
    softmax normalization); si-pairs share one es tile = the two DoubleRow
    K-tiles of the AV matmul
  - vT carries 64 all-ones columns -> the AV matmul emits the softmax
    denominator Z broadcast on partitions 64:128 of its PSUM tile
  - Z normalization: pairs 0-2 reshape Z to [128, 8] via a DRAM round trip for
    a cheap exact DVE reciprocal (off the critical path; an early copy releases
    the PSUM banks).  Pair 3 computes 1/Z = Exp(-Ln(Z)) on the then-idle ACT
    engine (ln+exp share an activation table), split by t-halves so the tail
    proj can chase it.
  - proj: pairs {0,1} and {2,3} are DoubleRow K-tile pairs; {0,1} runs between
    attn cores 2 and 3, {2,3} in the tail, each + bias/residual on DVE.
  - input DMAs: x tiles split across the 3 DMA-capable queues ahead of all
    weights; the vector queue stays DMA-free so GroupNorm starts ASAP.
"""

import numpy as np

B, C, HW, T = 8, 512, 32, 1024
H, CH = 8, 64
G, GS = 32, 16
EPS = 1e-5
NCORES = 8
LOG16 = float(np.log(16.0))

_CACHE = {}
TRACE = False  # test harness can set kernel.TRACE = True to get a profile


def _install_ntff_hook():
    import sys, types
    if 'antenv.axon_hooks' in sys.modules:
        return
    mod = types.ModuleType('antenv.axon_hooks')
    state = {'hook': None}
    mod.set_axon_ntff_profile_hook = lambda h: state.__setitem__('hook', h)
    mod.get_axon_ntff_profile_hook = lambda: state['hook']
    sys.modules['antenv.axon_hooks'] = mod
    import antenv
    antenv.axon_hooks = mod
    try:
        from trn_agent_boot.trn_boot import _ntff_profile_via_ctypes
        mod.set_axon_ntff_profile_hook(_ntff_profile_via_ctypes('/opt/axon/libaxon_pjrt.so'))
    except Exception:
        pass


def _split_multi_waits(nc, max_waits=1):
    """This container's walrus supports only one sync wait per instruction; move
    extra waits onto same-engine no-ops inserted just before the instruction."""
    import concourse.mybir as mybir
    for f in nc.m.functions:
        for bb in f.blocks:
            insts = bb.instructions
            out = []
            changed = False
            for inst in insts:
                si = inst.sync_info
                waits = list(si.on_wait) if si is not None and si.on_wait else []
                if len(waits) > max_waits:
                    changed = True
                    for j, w in enumerate(waits[:-max_waits]):
                        out.append(mybir.InstNoOp(
                            name=f"{inst.name}-ws{j}",
                            sync_info=mybir.SyncInfo(on_wait=[w], on_update=[]),
                            bass_nofuse=True,
                            engine=inst.engine,
                        ))
                    inst.sync_info = mybir.SyncInfo(
                        on_wait=waits[-max_waits:],
                        on_update=list(si.on_update) if si.on_update else [],
                    )
                out.append(inst)
            if changed:
                bb.instructions = out


def _build_nc():
    import concourse.bass as bass
    import concourse.tile as tile
    import concourse.mybir as mybir

    f32 = mybir.dt.float32
    f32r = mybir.dt.float32r
    f8 = mybir.dt.float8e4
    Alu = mybir.AluOpType
    Act = mybir.ActivationFunctionType
    DR = mybir.MatmulPerfMode.DoubleRow

    nc = bass.Bass()

    xin = nc.dram_tensor("xin", [C, T], f32, kind="ExternalInput")
    wqkT = nc.dram_tensor("wqkT", [C, 1024], f8, kind="ExternalInput")
    wvT8 = nc.dram_tensor("wvT8", [C, C], f8, kind="ExternalInput")
    wpT8 = nc.dram_tensor("wpT8", [C, C], f8, kind="ExternalInput")
    bqk = nc.dram_tensor("bqk", [1024], f32, kind="ExternalInput")
    bv = nc.dram_tensor("bv", [C], f32, kind="ExternalInput")
    bp = nc.dram_tensor("bp", [C], f32, kind="ExternalInput")
    gam = nc.dram_tensor("gam", [C], f32, kind="ExternalInput")
    bet = nc.dram_tensor("bet", [C], f32, kind="ExternalInput")
    maskA = nc.dram_tensor("maskA", [128, 8], f32r, kind="ExternalInput")
    maskB = nc.dram_tensor("maskB", [8, 128], f32r, kind="ExternalInput")
    outd = nc.dram_tensor("outd", [C, T], f32, kind="ExternalOutput")

    with tile.TileContext(nc) as tc:
        with tc.tile_pool(name="const", bufs=1) as const, \
             tc.tile_pool(name="big", bufs=1) as big, \
             tc.tile_pool(name="qpp", bufs=2) as qpp, \
             tc.tile_pool(name="kpp", bufs=4) as kpp, \
             tc.tile_pool(name="esp", bufs=6) as esp, \
             tc.tile_pool(name="accp", bufs=1) as accp, \
             tc.tile_pool(name="zp", bufs=2) as zp, \
             tc.tile_pool(name="gn", bufs=2) as gn, \
             tc.tile_pool(name="ps", bufs=2, space="PSUM") as ps, \
             tc.tile_pool(name="dram", bufs=2, space="DRAM") as dram:

            # ---- input DMAs.  x tiles first so GroupNorm can start ~5us in;
            # weights follow on the same queues.  The vector queue issues no
            # DMAs (it runs GroupNorm + evacuations). ----
            xt = [big.tile([128, 1024], f32, tag=f"x{ci}", name=f"x{ci}") for ci in range(4)]
            xr = xin.rearrange("(ci p) t -> p ci t", p=128)
            nc.sync.dma_start(out=xt[0], in_=xr[:, 0, :])
            nc.scalar.dma_start(out=xt[1], in_=xr[:, 1, :])
            nc.gpsimd.dma_start(out=xt[2], in_=xr[:, 2, :])
            nc.sync.dma_start(out=xt[3][:, 0:512], in_=xr[:, 3, 0:512])
            nc.scalar.dma_start(out=xt[3][:, 512:1024], in_=xr[:, 3, 512:1024])

            # tiny GN constants right behind x on sync/scalar (land ~x-time)
            gam_t = const.tile([128, 4], f32)
            nc.sync.dma_start(out=gam_t, in_=gam.rearrange("(ci p) -> p ci", p=128))
            bet_t = const.tile([128, 4], f32)
            nc.sync.dma_start(out=bet_t, in_=bet.rearrange("(ci p) -> p ci", p=128))
            mA = const.tile([128, 8], f32r)
            nc.scalar.dma_start(out=mA, in_=maskA[:, :])
            mB = const.tile([8, 128], f32r)
            nc.scalar.dma_start(out=mB, in_=maskB[:, :])
            eps_t = const.tile([128, 1], f32)
            nc.vector.memset(eps_t, EPS)
            nl16_t = const.tile([128, 1], f32)
            nc.vector.memset(nl16_t, -LOG16)

            # v weights next (v runs first after GN)
            wvT_t = const.tile([128, 4, 512], f8)
            wvr = wvT8.rearrange("(ci p) o -> p ci o", p=128)
            nc.sync.dma_start(out=wvT_t[:, 0:2, :], in_=wvr[:, 0:2, :])
            nc.scalar.dma_start(out=wvT_t[:, 2:4, :], in_=wvr[:, 2:4, :])
            # qk weights
            wqkT_t = const.tile([128, 4, 1024], f8)
            wqr = wqkT.rearrange("(ci p) o -> p ci o", p=128)
            weng = [nc.sync, nc.scalar, nc.gpsimd, nc.gpsimd]
            for ci in range(4):
                weng[ci].dma_start(out=wqkT_t[:, ci, :], in_=wqr[:, ci, :])
            # small biases
            bqk_t = const.tile([128, 8], f32)
            nc.gpsimd.dma_start(out=bqk_t, in_=bqk.rearrange("(oi p) -> p oi", p=128))
            bv_b = const.tile([128, 512], f32)
            nc.gpsimd.dma_start(out=bv_b, in_=bass.AP(
                tensor=bv, offset=0, ap=[[0, 128], [1, 512]]))
            bp_t = const.tile([128, 4], f32)
            nc.sync.dma_start(out=bp_t, in_=bp.rearrange("(ci p) -> p ci", p=128))
            # proj weights last (needed latest)
            wpT_t = const.tile([128, 4, 512], f8)
            wpr = wpT8.rearrange("(ci p) o -> p ci o", p=128)
            nc.sync.dma_start(out=wpT_t[:, 0:2, :], in_=wpr[:, 0:2, :])
            nc.scalar.dma_start(out=wpT_t[:, 2:4, :], in_=wpr[:, 2:4, :])

            # ---- GroupNorm, pipelined per 128-channel tile ----
            # each ci tile holds 8 complete groups (16 channels each), so the
            # whole stats -> group-reduce -> normalize chain runs per ci as
            # its x DMA lands, overlapping the remaining x/weight DMAs.
            # The tiny mask matmuls use PSUM tag "a" (idle until attention).
            ht = big.tile([128, 4, 1024], f8, tag="h", name="h")
            chmv = gn.tile([128, 4, 2], f32)
            s2ch = gn.tile([128, 4, 2], f32r)
            gf = gn.tile([8, 4, 2], f32r)
            mg = gn.tile([8, 4], f32)
            vg = gn.tile([8, 4], f32)
            m2 = gn.tile([8, 4], f32)
            scl = gn.tile([128, 4], f32)
            sht = gn.tile([128, 4], f32)
            pg = ps.tile([128, 1024], f32, tag="a", name="pgn")
            pc = ps.tile([128, 1024], f32, tag="a", name="pgc")
            for ci in range(4):
                st = gn.tile([128, 2, 6], f32, tag="st")
                xv = xt[ci].rearrange("p (n f) -> p n f", f=512)
                for sub in range(2):
                    nc.vector.bn_stats(out=st[:, sub, :], in_=xv[:, sub, :])
                nc.vector.bn_aggr(out=chmv[:, ci, :], in_=st)
                # per-channel [mean, var+mean^2] as f32r for the mask matmul
                nc.vector.tensor_copy(out=s2ch[:, ci, 0:1], in_=chmv[:, ci, 0:1])
                with nc.allow_low_precision(reason="f32r is f32 bits (matmul tag)"):
                    nc.vector.scalar_tensor_tensor(
                        out=s2ch[:, ci, 1:2], in0=chmv[:, ci, 0:1],
                        scalar=chmv[:, ci, 0:1], in1=chmv[:, ci, 1:2],
                        op0=Alu.mult, op1=Alu.add)
                # group means: maskA carries the 1/16 so pg = [mean, E[x^2]]
                nc.tensor.matmul(pg[0:8, 2 * ci:2 * ci + 2], lhsT=mA,
                                 rhs=s2ch[:, ci, :], start=True, stop=True)
                nc.vector.tensor_copy(out=mg[:, ci:ci + 1],
                                      in_=pg[0:8, 2 * ci:2 * ci + 1])
                nc.vector.tensor_mul(out=m2[:, ci:ci + 1],
                                     in0=pg[0:8, 2 * ci:2 * ci + 1],
                                     in1=mg[:, ci:ci + 1])
                nc.vector.tensor_sub(out=vg[:, ci:ci + 1],
                                     in0=pg[0:8, 2 * ci + 1:2 * ci + 2],
                                     in1=m2[:, ci:ci + 1])
                # rstd = 1/sqrt(vg + eps)
                nc.scalar.activation(out=vg[:, ci:ci + 1], in_=vg[:, ci:ci + 1],
                                     func=Act.Sqrt, bias=eps_t[:8], scale=1.0)
                with nc.allow_low_precision(reason="f32r is f32 bits (matmul tag)"):
                    nc.vector.reciprocal(out=gf[:, ci, 1:2], in_=vg[:, ci:ci + 1])
                nc.vector.tensor_copy(out=gf[:, ci, 0:1], in_=mg[:, ci:ci + 1])
                # broadcast group stats back to the tile's 128 channels
                nc.tensor.matmul(pc[:, 2 * ci:2 * ci + 2], lhsT=mB,
                                 rhs=gf[:, ci, :], start=True, stop=True)
                nc.vector.tensor_mul(out=scl[:, ci:ci + 1], in0=gam_t[:, ci:ci + 1],
                                     in1=pc[:, 2 * ci + 1:2 * ci + 2])
                nc.vector.tensor_mul(out=sht[:, ci:ci + 1], in0=scl[:, ci:ci + 1],
                                     in1=pc[:, 2 * ci:2 * ci + 1])
                nc.vector.tensor_sub(out=sht[:, ci:ci + 1], in0=bet_t[:, ci:ci + 1],
                                     in1=sht[:, ci:ci + 1])
                # h = x * scl + sht   (fp8, feeds the DoubleRow matmuls)
                nc.vector.tensor_scalar(out=ht[:, ci, :], in0=xt[ci],
                                        scalar1=scl[:, ci:ci + 1], scalar2=sht[:, ci:ci + 1],
                                        op0=Alu.mult, op1=Alu.add)

            # ---- vT[t, o_v]: tile tp is only needed by the AV step for
            # s-chunk pair tp, so tp>=1 is prefetched inside attn core 0 ----
            # cols 64:128 of each [t, head] block are all-ones: the AV matmul
            # then emits the softmax denominator Z broadcast across partitions
            # 64:128 of its PSUM tile (matmul cost only depends on N).
            vTa = big.tile([128, 8, 8, 128], f8)  # [t_part, ti, hd, ch+ones]
            nc.gpsimd.memset(vTa[:, :, :, 64:128], 1.0)

            def v_tp(tp):
                pv = ps.tile([128, 1024], f32, tag="s", name=f"pv{tp}")
                for half in range(2):
                    ti = 2 * tp + half
                    for g in range(2):
                        nc.tensor.matmul(pv[:, half * 512:(half + 1) * 512],
                                         lhsT=ht[:, 2 * g:2 * g + 2, ti * 128:(ti + 1) * 128],
                                         rhs=wvT_t[:, 2 * g:2 * g + 2, :],
                                         start=(g == 0), stop=(g == 1), perf_mode=DR)
                nc.vector.tensor_add(
                    out=vTa[:, 2 * tp:2 * tp + 2, :, 0:64],
                    in0=pv.rearrange("p (t h c) -> p t h c", t=2, h=8),
                    in1=bass.AP(tensor=bv_b.tensor, offset=bv_b.offset,
                                ap=[list(bv_b.ap[0])] + [[0, 2]] + [[64, 8], [1, 64]]))

            # ---- q,k for one head pair (f32r; scores keep full precision) ----
            qp = [None] * 4
            kpad = [None] * 8

            def qkv_side(pi, side):
                oi = side * 4 + pi
                pqk = ps.tile([128, 1024], f32, tag="s", name=f"pqk{oi}")
                for g in range(2):
                    for ni in range(2):
                        nc.tensor.matmul(
                            pqk[:, ni * 512:(ni + 1) * 512],
                            lhsT=wqkT_t[:, 2 * g:2 * g + 2, oi * 128:(oi + 1) * 128],
                            rhs=ht[:, 2 * g:2 * g + 2, ni * 512:(ni + 1) * 512],
                            start=(g == 0), stop=(g == 1), perf_mode=DR)
                if side == 0:
                    nc.vector.tensor_scalar_add(out=qp[pi], in0=pqk,
                                                scalar1=bqk_t[:, oi:oi + 1])
                else:
                    nc.vector.tensor_scalar_add(
                        out=kpad[2 * pi][0:64, :], in0=pqk[0:64, :],
                        scalar1=bqk_t[0:64, oi:oi + 1])
                    nc.vector.tensor_scalar_add(
                        out=kpad[2 * pi + 1][64:128, :], in0=pqk[64:128, :],
                        scalar1=bqk_t[64:128, oi:oi + 1])

            def qkv_alloc(pi):
                qp[pi] = qpp.tile([128, 1024], f32r, tag="qp", name=f"qp{pi}")
                kpad[2 * pi] = kpp.tile([128, 1024], f32r, tag="kp", name=f"kp{2*pi}")
                kpad[2 * pi + 1] = kpp.tile([128, 1024], f32r, tag="kp", name=f"kp{2*pi+1}")
                nc.gpsimd.memset(kpad[2 * pi][64:128, :].bitcast(f32), 0.0)
                nc.gpsimd.memset(kpad[2 * pi + 1][0:64, :].bitcast(f32), 0.0)

            def qkv_pair(pi):
                qkv_alloc(pi)
                qkv_side(pi, 0)
                qkv_side(pi, 1)

            # ---- attention (one head pair at a time) ----
            at_all = big.tile([128, 4, 1024], f8, tag="at", name="at")
            acc = [accp.tile([128, 1024], f32, tag=f"acc{oi}", name=f"acc{oi}")
                   for oi in range(4)]
            pa_all = [None] * 4

            def attn_core(pi, prefetch=()):
                """prefetch: {si: thunk} emitted after that si step's matmuls —
                used to overlap the next pair's qkv work with this pair's
                exp-bound pipeline (all prefetch deps must already be ready
                so the FIFO PSUM slot grants can't stall the score allocs)."""
                prefetch = dict(prefetch)
                pa = [ps.tile([128, 1024], f32, tag="a", name=f"pa{pi}_{i}") for i in range(2)]
                pa_all[pi] = pa
                es2 = [None, None]
                for si in range(8):
                    u, j = si // 2, si % 2
                    if j == 0:
                        es2 = [esp.tile([128, 2, 1024], f8, tag="es",
                                        name=f"es{pi}_{u}_{k}") for k in range(2)]
                    for th in prefetch.get(si, ()):
                        th()
                    pss = [ps.tile([128, 1024], f32, tag="s", name=f"pss{pi}_{si}_{i}")
                           for i in range(2)]
                    for half in range(2):
                        for ni in range(2):
                            nc.tensor.matmul(
                                pss[half][:, ni * 512:(ni + 1) * 512],
                                lhsT=kpad[2 * pi + half][:, si * 128:(si + 1) * 128],
                                rhs=qp[pi][:, ni * 512:(ni + 1) * 512],
                                start=True, stop=True)
                    for half in range(2):
                        # es = exp(S - ln16) as fp8; the si-pair shares one es
                        # tile = the two DoubleRow K-tiles of the AV matmul
                        nc.scalar.activation(out=es2[half][:, j, :], in_=pss[half],
                                             func=Act.Exp, bias=nl16_t, scale=1.0)
                    if j == 1:
                        for half in range(2):
                            hd = 2 * pi + half
                            for ni in range(2):
                                nc.tensor.matmul(
                                    pa[half][:, ni * 512:(ni + 1) * 512],
                                    lhsT=vTa[:, 2 * u:2 * u + 2, hd, :],
                                    rhs=es2[half][:, :, ni * 512:(ni + 1) * 512],
                                    start=(u == 0), stop=(u == 3), perf_mode=DR)

            def divide(pi, tail=False):
                """at[pi] = AV / Z.  Z arrives pre-broadcast on partitions
                64:128 of the AV PSUM tile (ones-columns in vTa).
                Exact DVE reciprocal costs ~4.7ns/free-elem, so the fast path
                reshapes Z to [128, 8] via a DRAM round trip (off the critical
                path; an early copy releases the PSUM banks).  The tail pair
                instead computes 1/Z = Exp(-Ln(Z)) on the then-idle ACT engine
                (ln+exp share an activation table), split by t-halves so the
                tail proj can chase it."""
                pa = pa_all[pi]
                if tail:
                    for ni in range(2):
                        sl = slice(ni * 512, (ni + 1) * 512)
                        for half in range(2):
                            lz = zp.tile([64, 1024], f32, tag="lz")
                            nc.scalar.activation(out=lz[:, sl], in_=pa[half][64:128, sl],
                                                 func=Act.Ln)
                            rzb = zp.tile([64, 1024], f32, tag="rz")
                            nc.scalar.activation(out=rzb[:, sl], in_=lz[:, sl],
                                                 func=Act.Exp, scale=-1.0)
                            nc.vector.tensor_tensor(
                                out=at_all[half * 64:half * 64 + 64, pi, sl],
                                in0=pa[half][0:64, sl], in1=rzb[:, sl], op=Alu.mult)
                    return
                aus = []
                for half in range(2):
                    au = zp.tile([65, 1024], f32, tag="au")
                    nc.vector.tensor_copy(out=au, in_=pa[half][0:65, :])
                    aus.append(au)
                for half in range(2):
                    eng = nc.sync if half == 0 else nc.gpsimd
                    zd = dram.tile([1, 1024], f32, tag="zd")
                    eng.dma_start(out=zd, in_=aus[half][64:65, :])
                    zq = zp.tile([128, 8], f32, tag="zq")
                    eng.dma_start(out=zq, in_=bass.AP(
                        tensor=zd.tensor, offset=zd.offset, ap=[[8, 128], [1, 8]]))
                    nc.vector.reciprocal(out=zq, in_=zq)
                    zd2 = dram.tile([1, 1024], f32, tag="zd2")
                    eng.dma_start(
                        out=bass.AP(tensor=zd2.tensor, offset=zd2.offset,
                                    ap=[[8, 128], [1, 8]]), in_=zq)
                    zb = zp.tile([64, 1024], f32, tag="zb")
                    eng.dma_start(out=zb, in_=bass.AP(
                        tensor=zd2.tensor, offset=zd2.offset, ap=[[0, 64], [1, 1024]]))
                    nc.vector.tensor_tensor(
                        out=at_all[half * 64:half * 64 + 64, pi, :],
                        in0=aus[half][0:64, :], in1=zb, op=Alu.mult)

            def proj01(ois):
                """pairs {0,1}: one DoubleRow pass per (oi, ni) + bias/residual."""
                for oi in ois:
                    pp = ps.tile([128, 1024], f32, tag="s", name=f"pp01_{oi}")
                    for ni in range(2):
                        nc.tensor.matmul(pp[:, ni * 512:(ni + 1) * 512],
                                         lhsT=wpT_t[:, 0:2, oi * 128:(oi + 1) * 128],
                                         rhs=at_all[:, 0:2, ni * 512:(ni + 1) * 512],
                                         start=True, stop=True, perf_mode=DR)
                    nc.vector.scalar_tensor_tensor(
                        out=acc[oi], in0=pp, scalar=bp_t[:, oi:oi + 1],
                        in1=xt[oi], op0=Alu.add, op1=Alu.add)

            def proj_tail():
                """pairs {2,3}: per output block one DoubleRow pass per ni ->
                add -> DMA out, pipelined behind the t-half-split divide(3)."""
                oeng = [nc.sync, nc.scalar, nc.gpsimd, nc.sync]
                outr = outd.rearrange("(ci p) t -> p ci t", p=128)
                for oi in range(4):
                    pp = ps.tile([128, 1024], f32, tag=("a" if oi >= 2 else "s"),
                                 name=f"ppt_{oi}")
                    for ni in range(2):
                        nc.tensor.matmul(pp[:, ni * 512:(ni + 1) * 512],
                                         lhsT=wpT_t[:, 2:4, oi * 128:(oi + 1) * 128],
                                         rhs=at_all[:, 2:4, ni * 512:(ni + 1) * 512],
                                         start=True, stop=True, perf_mode=DR)
                    nc.vector.tensor_add(out=acc[oi], in0=acc[oi], in1=pp)
                    oeng[oi].dma_start(out=outr[:, oi, :], in_=acc[oi])

            qkv_pair(0)
            v_tp(0)
            qkv_alloc(1)
            attn_core(0, prefetch={1: [lambda: v_tp(1)],
                                   2: [lambda: qkv_side(1, 0)],
                                   3: [lambda: v_tp(2)],
                                   5: [lambda: v_tp(3)],
                                   6: [lambda: qkv_side(1, 1)]})
            divide(0)
            qkv_alloc(2)
            attn_core(1, prefetch={3: [lambda: qkv_side(2, 0)],
                                   5: [lambda: qkv_side(2, 1)]})
            divide(1)
            qkv_alloc(3)
            attn_core(2, prefetch={3: [lambda: qkv_side(3, 0)],
                                   5: [lambda: qkv_side(3, 1)]})
            divide(2)
            attn_core(3, prefetch={1: [lambda: proj01((0, 1))],
                                   3: [lambda: proj01((2, 3))]})
            divide(3, tail=True)
            proj_tail()

    _split_multi_waits(nc)
    return nc


def _prep_host(x, gamma, beta, w_qkv, b_qkv, w_proj, b_proj):
    """Host-side weight permutation/pre-scaling + per-core input maps."""
    import ml_dtypes
    x = np.ascontiguousarray(x, dtype=np.float32).reshape(B, C, T)
    scale2 = 1.0 / np.sqrt(CH)  # folded into q (exact: 0.125 is a power of two)

    w = np.asarray(w_qkv, dtype=np.float32).reshape(H, 3, CH, C)
    bq = np.asarray(b_qkv, dtype=np.float32).reshape(H, 3, CH)
    wq = w[:, 0] * scale2          # [hd, 64, c]
    wk = w[:, 1]
    wv = w[:, 2]
    # o-block order: 4 q-blocks (one per head pair: [q_{2i}; q_{2i+1}]), 4 k-blocks
    qcols = wq.reshape(4, 128, C).transpose(2, 0, 1)            # [c, pi, 128]
    kcols = wk.reshape(4, 128, C).transpose(2, 0, 1)
    wqkT_host = np.ascontiguousarray(
        np.concatenate([qcols, kcols], axis=1).reshape(C, 1024)
    ).astype(ml_dtypes.float8_e4m3)
    bqk_host = np.ascontiguousarray(np.concatenate(
        [(bq[:, 0] * scale2).reshape(4, 128), bq[:, 1].reshape(4, 128)], axis=0
    ).reshape(1024))
    wvT_host = np.ascontiguousarray(
        wv.transpose(2, 0, 1).reshape(C, C)).astype(ml_dtypes.float8_e4m3)
    bv_host = np.ascontiguousarray(bq[:, 2].reshape(C))
    wpT_host = np.ascontiguousarray(
        np.asarray(w_proj, dtype=np.float32).T).astype(ml_dtypes.float8_e4m3)
    maskA_host = np.zeros((128, 8), np.float32)
    for p in range(128):
        maskA_host[p, p // 16] = 1.0 / GS  # fold the group-mean 1/16 in
    maskB_host = np.ascontiguousarray(np.sign(maskA_host).T)

    common = {
        "wqkT": wqkT_host, "wvT8": wvT_host, "wpT8": wpT_host,
        "bqk": bqk_host, "bv": bv_host,
        "bp": np.ascontiguousarray(np.asarray(b_proj, np.float32)),
        "gam": np.ascontiguousarray(np.asarray(gamma, np.float32)),
        "bet": np.ascontiguousarray(np.asarray(beta, np.float32)),
        "maskA": maskA_host, "maskB": maskB_host,
    }
    return [dict(common, xin=np.ascontiguousarray(x[b])) for b in range(B)]


def kernel(x, gamma, beta, w_qkv, b_qkv, w_proj, b_proj):
    from concourse.bass_utils import run_bass_kernel_spmd

    if "nc" not in _CACHE:
        _CACHE["nc"] = _build_nc()
    nc = _CACHE["nc"]

    in_maps = _prep_host(x, gamma, beta, w_qkv, b_qkv, w_proj, b_proj)
    kwargs = {}
    if TRACE:
        _install_ntff_hook()
        kwargs["trace"] = True
    res = run_bass_kernel_spmd(nc, in_maps, core_ids=list(range(NCORES)), **kwargs)
    if TRACE:
        _CACHE["last_result"] = res
    out = np.stack([r["outd"] for r in res.results], axis=0)
    return out.reshape(B, C, HW, HW)
